# revision 19
# baseline (speedup 1.0000x reference)
"""GQA attention (B=2, S=2048, D=2048, 32 Q heads / 8 KV heads, HD=64) on 8 trn2 cores.

Sharding: tensor-parallel over heads. Core c gets Q heads [4c, 4c+4), KV head c.
Each core computes a full [B*S, D] partial of the output (its 4 heads through
o_proj); the host sums the 8 partials. No collectives.

v2 design (vs the v1 baseline):
  - K and V projections merged into one matmul stream (stationary [wk|wv]
    [128,128]) -> K^T rows 0-63, V^T rows 64-127 of each PSUM tile. V^T is
    turned into natural V via PE transposes (the v1 h-stationary V projection
    was Ldweights-bound).
  - Scores matmuls are row-tiled pairs: contraction is HD=64, so two key
    blocks' K^T stationaries sit on partition halves (kt2[0:64]=even kb,
    kt2[64:128]=odd kb) and the two matmuls run CONCURRENTLY in the PE array
    (tile_position row groups, auto-derived from base partitions). Q is
    duplicated onto both partition halves via SBUF->SBUF DMA.
  - Normalization is job-level and decoupled: PV accumulators (PSUM) are
    drained per head to SBUF by one DVE copy, then recip -> gpsimd
    partition_broadcast -> DVE muls happen off the critical PE path
    (outp bufs=2 so the next head's PV never waits).
  - o_proj of job J is interleaved into job J+1's kb loop (and into the
    batch-1 projection phase) so the PE never idles while ACT does exp.
  - Output is written bf16 (host accumulates partials in f64).
"""

import functools

import numpy as np
import ml_dtypes

import concourse.bacc as bacc
import concourse.bass as bass
import concourse.mybir as mybir
import concourse.tile as tile
from concourse import masks
from concourse.bass_utils import run_bass_kernel_spmd

B, S, D = 2, 2048, 2048
H, KVH, HD = 32, 8, 64
NCORES = 8
QH = H // NCORES            # 4 q heads per core
ST = B * S                  # 4096 flattened rows
QHD = QH * HD               # 256 (q hd dims per core)
SCALE = 1.0 / np.sqrt(HD)

BF16 = mybir.dt.bfloat16
F32 = mybir.dt.float32

DC = D // 128               # 16 contraction chunks
SC_N = S // 512             # 4 s-chunks per batch for projections
KB_N = S // 128             # 16 key blocks per batch
KP_N = KB_N // 2            # 8 key-block pairs
QJ_N = S // 512             # 4 q-jobs of 512 per batch


def _rebalance_matmul_waits(nc):
    """walrus allows only one sync-wait on a Matmult. Tile occasionally emits
    two (psum-slot release + engine ordering) on the first matmul of an
    accumulation group. The dedicated Ldweights directly preceding the matmul
    runs on the same in-order PE queue and virtually never carries a wait, so
    shifting the surplus waits onto it preserves ordering semantics."""
    for fn in nc.m.functions:
        for blk in fn.blocks:
            insts = list(blk.instructions)
            for idx, inst in enumerate(insts):
                if type(inst).__name__ != "InstMatmult":
                    continue
                si = inst.sync_info
                waits = list(si.on_wait or []) if si else []
                if len(waits) <= 1:
                    continue
                prev = insts[idx - 1] if idx else None
                assert prev is not None and type(prev).__name__ == "InstLdweights", (
                    f"matmul {inst.name} has {len(waits)} waits but no "
                    f"preceding Ldweights (got {type(prev).__name__})")
                _shift_waits(inst, si, waits, prev)


def _shift_waits(inst, si, waits, carrier):
    psi = carrier.sync_info
    pwaits = list(psi.on_wait or []) if psi else []
    assert len(pwaits) + len(waits) - 1 <= 3, (
        f"{inst.name}: too many combined waits on carrier {carrier.name}")
    moved, kept = waits[:-1], waits[-1:]
    if psi is None:
        carrier.sync_info = type(si)(on_wait=moved, on_update=[])
    else:
        psi.on_wait = pwaits + moved
    si.on_wait = kept


def _rebalance_dma_waits(nc):
    """Same single-wait limit applies to HWDGE DMACopy / gpsimd DMA-direct
    instructions. These always read an SBUF tile written by a producer
    (DVE copy / reciprocal) a few instructions earlier; the producer's
    engine tolerates 3 waits, and since the DMA already waits on the
    producer, conditions moved onto the producer still hold when the DMA
    starts."""
    for fn in nc.m.functions:
        for blk in fn.blocks:
            insts = list(blk.instructions)
            sp_seen = {}   # sem name -> max value already awaited on SP queue
            for idx, inst in enumerate(insts):
                if type(inst).__name__ not in (
                        "InstDMACopy", "InstPartitionBroadcast"):
                    continue
                si = inst.sync_info
                waits = list(si.on_wait or []) if si else []
                is_sp = str(inst.engine) == "EngineType.SP"
                if is_sp and waits:
                    # SP executes serially: waits dominated by an earlier SP
                    # instruction's wait on the same sem are redundant
                    live = [w for w in waits
                            if sp_seen.get(w.ant_name, -1) < w.wait_value]
                    if len(live) < len(waits):
                        si.on_wait = live
                        waits = live
                if is_sp:
                    for w in waits:
                        if sp_seen.get(w.ant_name, -1) < w.wait_value:
                            sp_seen[w.ant_name] = w.wait_value
                if len(waits) <= 1:
                    continue
                src = inst.ins[0].memref if inst.ins else None
                prod = None
                for j in range(idx - 1, max(-1, idx - 400), -1):
                    p = insts[j]
                    pouts = getattr(p, "outs", None)
                    if pouts and pouts[0].memref == src and \
                            type(p).__name__ not in ("InstDMACopy",):
                        prod = p
                        break
                if prod is None:
                    # DRAM load: no producer. SP executes serially, so the
                    # nearest preceding wait-free SP DMA can absorb the
                    # engine-WAR wait; the queue wait stays on this DMA.
                    carrier = None
                    for j in range(idx - 1, max(-1, idx - 400), -1):
                        p = insts[j]
                        if type(p).__name__ == "InstDMACopy" and \
                                str(p.engine) == "EngineType.SP":
                            pw = list(p.sync_info.on_wait or []) \
                                if p.sync_info else []
                            if not pw:
                                carrier = p
                                break
                    if carrier is None:
                        # The engine-WAR wait (kept) implies the slot's
                        # previous DMA write completed (its readers waited on
                        # it), so the same-queue WAW wait is redundant.
                        keep = [w for w in waits if "DMAHW" not in w.ant_name]
                        assert len(keep) == 1, (
                            f"{inst.name}: unexpected pair "
                            f"{[(w.ant_name, w.wait_value) for w in waits]}")
                        si.on_wait = keep
                        continue
                    waits.sort(key=lambda w: 1 if "DMAHW" in w.ant_name else 0)
                    _shift_waits(inst, si, waits, carrier)
                    continue
                # keep the producer-engine wait on the DMA, move the rest
                eng = str(prod.engine)
                key = {"EngineType.DVE": "DVE", "EngineType.ACT": "Activation",
                       "EngineType.Pool": "Pool", "EngineType.PE": "PE",
                       "EngineType.SP": "SP"}.get(eng, "zz")
                waits.sort(key=lambda w: 0 if w.ant_name.startswith(key) else 1)
                waits = waits[::-1]  # producer wait last -> kept
                psi = prod.sync_info
                pn = len(list(psi.on_wait or [])) if psi else 0
                if pn + len(waits) - 1 <= 3:
                    _shift_waits(inst, si, waits, prod)
                else:
                    # producer full: queue wait is FIFO-covered (slot reuse
                    # distance is a multiple of the 8 round-robin queues)
                    keep = [w for w in waits if "DMAHW" not in w.ant_name]
                    assert len(keep) == 1, (
                        f"{inst.name}: unexpected {[(w.ant_name, w.wait_value) for w in waits]}")
                    si.on_wait = keep


def _pin_act_tables(nc):
    """The act-table pass picks the first table containing each activation's
    function, so a kernel using Exp and Ln thrashes between table 0
    (exp_and_others) and table 5 (natural_log, which lacks exp) — one
    1.28us ACT table load per job boundary. All functions this kernel uses
    (Exp, Ln, Copy) live together in 'natural_log_exp_and_others', so pin
    the first load to that table and drop the rest (they carry no waits or
    semaphore updates)."""
    from concourse.hw_specs import get_activation_tables

    tables = get_activation_tables(nc.m.arch)
    nl_id = list(tables).index("natural_log_exp_and_others")
    fns = tables["natural_log_exp_and_others"]
    for need in (mybir.ActivationFunctionType.Exp,
                 mybir.ActivationFunctionType.Ln,
                 mybir.ActivationFunctionType.Copy):
        assert need in fns, need
    for fn in nc.m.functions:
        for blk in fn.blocks:
            first = True
            kept = []
            for inst in blk.instructions:
                if isinstance(inst, mybir.InstLoadActFuncSet):
                    si = inst.sync_info
                    assert not (si and (si.on_wait or si.on_update)), inst.name
                    if not first:
                        continue
                    inst.act_func_set_id = nl_id
                    first = False
                kept.append(inst)
            blk.instructions[:] = kept


def build_program(trace_friendly: bool = False):
    nc = bacc.Bacc("TRN2", target_bir_lowering=False)
    ht = nc.dram_tensor("ht", [D, ST], BF16, kind="ExternalInput")
    wq = nc.dram_tensor("wq", [D, QHD], BF16, kind="ExternalInput")
    wkv = nc.dram_tensor("wkv", [D, 2 * HD], BF16, kind="ExternalInput")
    wo = nc.dram_tensor("wo", [QHD, D], BF16, kind="ExternalInput")
    out = nc.dram_tensor("out", [ST, D], BF16, kind="ExternalOutput")

    with tile.TileContext(nc) as tc:
        with (
            tc.tile_pool(name="singles", bufs=1) as singles,
            tc.tile_pool(name="hstream", bufs=3) as hstream,
            tc.tile_pool(name="expp", bufs=4) as expp,
            tc.tile_pool(name="araw", bufs=2) as arawp,
            tc.tile_pool(name="attn", bufs=2) as attnp,
            tc.tile_pool(name="norm", bufs=2) as normp,
            tc.tile_pool(name="ostage", bufs=4) as ostage,
            tc.tile_pool(name="ps_sc", bufs=2, space="PSUM") as ps_sc,
            tc.tile_pool(name="ps_out", bufs=1, space="PSUM") as ps_out,
            tc.tile_pool(name="ps_op", bufs=2, space="PSUM") as ps_op,
        ):
            # ---- resident weights ----
            # Load order matters for the lead-in: wkv (needed by the first
            # matmul) goes first; the first h chunk is DMA'd right after in
            # proj_phase; wq follows; wo is only needed once the first
            # o_proj group runs (one full job later), so it loads last.
            wq_sb = singles.tile([128, DC, QHD], BF16)
            wkv_sb = singles.tile([128, DC, 2 * HD], BF16)
            wo_sb = singles.tile([128, 2, D], BF16)
            nc.sync.dma_start(
                wkv_sb[:, :, :],
                wkv[:, :].rearrange("(a p) j -> p a j", p=128))

            # identity for the PE V-transposes
            id_sb = singles.tile([64, HD], BF16)
            masks.make_identity(nc, id_sb[:, :])

            # ---- resident activations (per batch) ----
            # qtdup[h][b]: [128, S], Q^T duplicated on both partition halves
            qtdup = [[singles.tile([128, S], BF16, tag=f"qt{h}_{b}",
                                   name=f"qt{h}_{b}")
                      for b in range(B)] for h in range(QH)]
            # kt2[b]: [128, KP_N, 128]; rows 0:64 = even kb K^T, 64:128 = odd
            kt2 = [singles.tile([128, KP_N, 128], BF16, tag=f"kt{b}",
                                name=f"kt{b}") for b in range(B)]
            vaug = [singles.tile([128, KB_N, HD + 1], BF16, tag=f"vaug{b}",
                                 name=f"vaug{b}") for b in range(B)]
            # V^T staging: rows 64:128 written by DVE (in-partition from
            # PSUM), rows 0:64 filled by SBUF->SBUF DMA; transposed at the
            # end of the projection phase.
            vt_sb = [singles.tile([128, S], BF16, tag=f"vt{b}", name=f"vt{b}")
                     for b in range(B)]
            for b in range(B):
                nc.vector.memset(vaug[b][:, :, HD:HD + 1], 1.0)

            # pending o_proj work from the previous attention job: a list of
            # closures, each one (2 matmuls + a DVE cast [+ DMA]).
            pending = []

            def run_pending(n):
                for _ in range(min(n, len(pending))):
                    pending.pop(0)()

            def proj_phase(b, pieces=None, split=False):
                """Emit batch-b projection work as ~1.7us closures.

                pieces=None: everything inline. split=True (with pieces):
                KV + Q-heads-0/1 + V-transposes run inline, Q-heads-2/3 are
                appended to pieces (their h chunks are re-loaded, trading
                ~8.4MB of DMA reads to start attention ~20us earlier).
                split=False with pieces: everything is appended."""
                inline = lambda f: f()
                emit = inline if pieces is None else pieces.append
                h_tiles = {}
                pkv_t = {}
                pq_t = {}

                def load(sc, p2=False, b=b):
                    if sc >= SC_N:
                        return
                    h_sb = hstream.tile([128, DC, 512], BF16, tag="h")
                    h_tiles[(p2, sc)] = h_sb
                    scol = b * S + sc * 512
                    # two halves so the first matmuls start at ~1MB loaded
                    for hf in range(2):
                        nc.sync.dma_start(
                            h_sb[:, 8 * hf:8 * hf + 8, :],
                            ht[1024 * hf:1024 * hf + 1024,
                               scol:scol + 512].rearrange(
                                "(a p) j -> p a j", p=128))

                def kv_a(sc, b=b):
                    load(sc + 1)
                    h_sb = h_tiles[(False, sc)]
                    pkv = ps_op.tile([128, 512], F32, tag="po")
                    pkv_t[sc] = pkv
                    for dc in range(8):
                        nc.tensor.matmul(pkv, wkv_sb[:, dc], h_sb[:, dc],
                                         start=(dc == 0), stop=False)

                def kv_b(sc, b=b):
                    h_sb = h_tiles[(False, sc)]
                    pkv = pkv_t.pop(sc)
                    scol = sc * 512
                    for dc in range(8, DC):
                        nc.tensor.matmul(pkv, wkv_sb[:, dc], h_sb[:, dc],
                                         start=False, stop=(dc == DC - 1))
                    # K^T rows 0:64 -> kt2: kb 4sc+j; even j -> low half
                    # (in-partition), odd j -> high half (cross-partition)
                    for j in range(4):
                        kp, half = (4 * sc + j) // 2, (4 * sc + j) % 2
                        nc.vector.tensor_copy(
                            kt2[b][64 * half:64 * half + 64, kp, :],
                            pkv[0:64, j * 128:(j + 1) * 128])
                    # V^T rows 64:128 -> staging high half, then DMA down
                    nc.vector.tensor_copy(
                        vt_sb[b][64:128, scol:scol + 512], pkv[64:128, :])
                    nc.sync.dma_start(
                        vt_sb[b][0:64, scol:scol + 512],
                        vt_sb[b][64:128, scol:scol + 512])

                def q_a(sc, m, p2, b=b):
                    h_sb = h_tiles[(p2, sc)]
                    pq = ps_op.tile([128, 512], F32, tag="po")
                    pq_t[(sc, m)] = pq
                    for dc in range(8):
                        nc.tensor.matmul(
                            pq, wq_sb[:, dc, m * 128:(m + 1) * 128],
                            h_sb[:, dc], start=(dc == 0), stop=False)

                def q_b(sc, m, p2, b=b):
                    scol = sc * 512
                    h_sb = h_tiles[(p2, sc)]
                    if p2:
                        load(sc + 2, p2=True)
                    pq = pq_t.pop((sc, m))
                    for dc in range(8, DC):
                        nc.tensor.matmul(
                            pq, wq_sb[:, dc, m * 128:(m + 1) * 128],
                            h_sb[:, dc], start=False, stop=(dc == DC - 1))
                    h0, h1 = 2 * m, 2 * m + 1
                    nc.vector.tensor_copy(
                        qtdup[h0][b][0:64, scol:scol + 512], pq[0:64, :])
                    nc.vector.tensor_copy(
                        qtdup[h1][b][64:128, scol:scol + 512], pq[64:128, :])
                    # duplicate onto the other partition half (DMA)
                    nc.sync.dma_start(
                        qtdup[h0][b][64:128, scol:scol + 512],
                        qtdup[h0][b][0:64, scol:scol + 512])
                    nc.sync.dma_start(
                        qtdup[h1][b][0:64, scol:scol + 512],
                        qtdup[h1][b][64:128, scol:scol + 512])

                def vtrans(g, b=b):
                    tr = ps_op.tile([128, 4, HD], BF16, tag="po")
                    for j in range(4):
                        kb = 4 * g + j
                        nc.tensor.transpose(
                            tr[:, j, :],
                            vt_sb[b][0:64, kb * 128:(kb + 1) * 128],
                            id_sb[:, :])
                    nc.vector.tensor_copy(
                        vaug[b][:, 4 * g:4 * g + 4, 0:HD], tr[:, :, :])

                load(0)
                if split:
                    assert pieces is not None
                    for sc in range(SC_N):
                        if sc == SC_N - 1:
                            # fired here (not earlier) so the hstream ring
                            # never makes a pass1 load wait on pass2 readers
                            load(0, p2=True)
                        kv_a(sc), kv_b(sc)
                        q_a(sc, 0, False), q_b(sc, 0, False)
                    for g in range(4):
                        vtrans(g)
                    load(1, p2=True)
                    for sc in range(SC_N):
                        pieces.append(functools.partial(q_a, sc, 1, True))
                        pieces.append(functools.partial(q_b, sc, 1, True))
                else:
                    for sc in range(SC_N):
                        emit(functools.partial(kv_a, sc))
                        emit(functools.partial(kv_b, sc))
                        for m in range(2):
                            emit(functools.partial(q_a, sc, m, False))
                            emit(functools.partial(q_b, sc, m, False))
                    for g in range(4):
                        emit(functools.partial(vtrans, g))

            # ---------- flat software-pipelined attention ----------
            pieces = []
            cur = {"outp": None, "araw": None, "expT": {}}

            def emit_scores(s):
                b, qj, h, kp = s
                q0 = qj * 512
                scp = ps_sc.tile([128, 1024], F32)
                nc.tensor.matmul(
                    scp[:, 0:512], kt2[b][0:64, kp, :],
                    qtdup[h][b][0:64, q0:q0 + 512], start=True, stop=True)
                nc.tensor.matmul(
                    scp[:, 512:1024], kt2[b][64:128, kp, :],
                    qtdup[h][b][64:128, q0:q0 + 512], start=True, stop=True)
                expT = expp.tile([128, 1024], BF16)
                nc.scalar.activation(expT[:, :], scp[:, :],
                                     mybir.ActivationFunctionType.Exp,
                                     scale=SCALE)
                cur["expT"][s] = expT

            def finish_job(b, qj, araw, last):
                # job-level normalization: 1/den as exp(-ln(den)) on ACT
                # (DVE reciprocal is ~6.3 ns/elem on one partition).
                q0 = qj * 512
                lnv = normp.tile([1, QH * 512], F32, tag="lnv")
                nc.scalar.activation(lnv, araw[64:65, :, :],
                                     mybir.ActivationFunctionType.Ln)
                recip = normp.tile([1, QH * 512], BF16, tag="recip")
                nc.scalar.activation(recip, lnv,
                                     mybir.ActivationFunctionType.Exp,
                                     scale=-1.0)
                bcast = normp.tile([64, QH * 512], BF16, tag="bcast")
                nc.gpsimd.partition_broadcast(bcast, recip)
                attn_sb = attnp.tile([128, 2, 512], BF16)
                for h in range(QH):
                    nc.vector.tensor_mul(
                        attn_sb[(h % 2) * 64:(h % 2) * 64 + 64, h // 2, :],
                        araw[0:64, h, :], bcast[:, h * 512:(h + 1) * 512])
                for qc in range(4):
                    ost = ostage.tile([128, 2048], BF16)
                    row = b * S + q0 + qc * 128
                    for nb in range(4):
                        def grp(qc=qc, nb=nb, ost=ost, row=row,
                                attn_sb=attn_sb, act_cast=(last and nb % 2)):
                            po = ps_op.tile([128, 512], F32, tag="po")
                            for hh in range(2):
                                nc.tensor.matmul(
                                    po,
                                    attn_sb[:, hh, qc * 128:(qc + 1) * 128],
                                    wo_sb[:, hh, nb * 512:(nb + 1) * 512],
                                    start=(hh == 0), stop=(hh == 1))
                            if act_cast:
                                nc.scalar.copy(
                                    ost[:, nb * 512:(nb + 1) * 512], po)
                            else:
                                nc.vector.tensor_copy(
                                    ost[:, nb * 512:(nb + 1) * 512], po)
                            if nb == 3:
                                for dd in range(2):
                                    nc.sync.dma_start(
                                        out[row:row + 128,
                                            dd * 1024:(dd + 1) * 1024],
                                        ost[:, dd * 1024:(dd + 1) * 1024])
                        pending.append(grp)

            def emit_pv(s, last):
                b, qj, h, kp = s
                expT = cur["expT"].pop(s)
                if kp == 0:
                    cur["outp"] = ps_out.tile([HD + 1, 2, 512], F32,
                                              name="pvab", tag="pvab")
                    if h == 0:
                        cur["araw"] = arawp.tile([65, QH, 512], BF16, name="araw", tag="araw")
                outp = cur["outp"]
                # row-tiled PV: key-halves of each kb run concurrently in
                # the PE array (tile rows 0-63 / 64-127), accumulating into
                # two separate banks; the drain adds them (and the two
                # denominator halves) back together.
                for kb2 in (2 * kp, 2 * kp + 1):
                    ex = expT[:, 0:512] if kb2 == 2 * kp else expT[:, 512:1024]
                    nc.tensor.matmul(outp[:, 0, :], vaug[b][0:64, kb2, :],
                                     ex[0:64, :],
                                     start=(kp == 0 and kb2 == 2 * kp),
                                     stop=(kp == KP_N - 1 and kb2 != 2 * kp))
                    nc.tensor.matmul(outp[:, 1, :], vaug[b][64:128, kb2, :],
                                     ex[64:128, :],
                                     start=(kp == 0 and kb2 == 2 * kp),
                                     stop=(kp == KP_N - 1 and kb2 != 2 * kp))
                if kp == KP_N - 1:
                    # drain this head's accumulator (frees PSUM fast).
                    # DVE may read only one PSUM operand per instruction, so
                    # copy bank A out then add bank B in place.
                    nc.vector.tensor_copy(cur["araw"][:, h, :], outp[:, 0, :])
                    nc.vector.tensor_add(cur["araw"][:, h, :],
                                         cur["araw"][:, h, :], outp[:, 1, :])
                    if h == QH - 1:
                        finish_job(b, qj, cur["araw"], last)
                        if last:
                            run_pending(len(pending))
                if kp in (2, 4, 7) and pieces:
                    pieces.pop(0)()
                if (h == 0 and kp >= 4) or (h > 0 and kp % 2 == 1):
                    run_pending(1)

            # ================= schedule =================
            # wq rides behind wkv + the first h chunk; wo (first needed by
            # o_proj of job (0,0), a full job later) loads after proj-b0.
            nc.sync.dma_start(
                wq_sb[:, :, :], wq[:, :].rearrange("(a p) j -> p a j", p=128))
            proj_phase(0, pieces=pieces, split=True)
            nc.sync.dma_start(
                wo_sb[:, :, :], wo[:, :].rearrange("(a p) j -> p a j", p=128))
            proj_phase(1, pieces=pieces)

            slots = [(b, qj, h, kp)
                     for b in range(B) for qj in range(QJ_N)
                     for h in range(QH) for kp in range(KP_N)]
            last_slot = slots[-1]
            prev = None
            for s in slots:
                if s[:3] == (1, 0, 0) and s[3] == 0:
                    assert not pieces, len(pieces)
                emit_scores(s)
                if prev is not None:
                    emit_pv(prev, last=False)
                prev = s
            emit_pv(prev, last=True)
            run_pending(len(pending))
    nc.compile()
    _pin_act_tables(nc)
    _rebalance_matmul_waits(nc)
    _rebalance_dma_waits(nc)
    return nc


@functools.lru_cache(maxsize=1)
def _get_program():
    return build_program()


def _in_maps(hidden_states, Wq, Wk, Wv, Wo):
    bf = ml_dtypes.bfloat16
    htT = np.ascontiguousarray(
        hidden_states.reshape(ST, D).T.astype(bf))          # [D, B*S]
    in_maps = []
    for c in range(NCORES):
        wkv = np.concatenate(
            [Wk[:, c * HD:(c + 1) * HD], Wv[:, c * HD:(c + 1) * HD]], axis=1)
        in_maps.append({
            "ht": htT,
            "wq": np.ascontiguousarray(Wq[:, c * QHD:(c + 1) * QHD].astype(bf)),
            "wkv": np.ascontiguousarray(wkv.astype(bf)),
            "wo": np.ascontiguousarray(Wo[c * QHD:(c + 1) * QHD, :].astype(bf)),
        })
    return in_maps


def kernel(hidden_states, Wq, Wk, Wv, Wo):
    hidden_states = np.asarray(hidden_states)
    Wq, Wk, Wv, Wo = (np.asarray(x) for x in (Wq, Wk, Wv, Wo))
    in_maps = _in_maps(hidden_states, Wq, Wk, Wv, Wo)
    nc = _get_program()
    res = run_bass_kernel_spmd(nc, in_maps, core_ids=list(range(NCORES)))
    total = res.results[0]["out"].astype(np.float64)
    for c in range(1, NCORES):
        total += res.results[c]["out"].astype(np.float64)
    return total.reshape(B, S, D).astype(np.float32)


# revision 20
# speedup vs baseline: 1.0672x; 1.0672x over previous
"""GQA attention (B=2, S=2048, D=2048, 32 Q heads / 8 KV heads, HD=64) on 8 trn2 cores.

Sharding: tensor-parallel over heads. Core c gets Q heads [4c, 4c+4), KV head c.
Each core computes a full [B*S, D] partial of the output (its 4 heads through
o_proj); the host sums the 8 partials. No collectives.

v2 design (vs the v1 baseline):
  - K and V projections merged into one matmul stream (stationary [wk|wv]
    [128,128]) -> K^T rows 0-63, V^T rows 64-127 of each PSUM tile. V^T is
    turned into natural V via PE transposes (the v1 h-stationary V projection
    was Ldweights-bound).
  - Scores matmuls are row-tiled pairs: contraction is HD=64, so two key
    blocks' K^T stationaries sit on partition halves (kt2[0:64]=even kb,
    kt2[64:128]=odd kb) and the two matmuls run CONCURRENTLY in the PE array
    (tile_position row groups, auto-derived from base partitions). Q is
    duplicated onto both partition halves via SBUF->SBUF DMA.
  - Normalization is job-level and decoupled: PV accumulators (PSUM) are
    drained per head to SBUF by one DVE copy, then recip -> gpsimd
    partition_broadcast -> DVE muls happen off the critical PE path
    (outp bufs=2 so the next head's PV never waits).
  - o_proj of job J is interleaved into job J+1's kb loop (and into the
    batch-1 projection phase) so the PE never idles while ACT does exp.
  - Output is written bf16 (host accumulates partials in f64).
"""

import functools

import numpy as np
import ml_dtypes

import concourse.bacc as bacc
import concourse.bass as bass
import concourse.mybir as mybir
import concourse.tile as tile
from concourse import masks
from concourse.bass_utils import run_bass_kernel_spmd

B, S, D = 2, 2048, 2048
H, KVH, HD = 32, 8, 64
NCORES = 8
QH = H // NCORES            # 4 q heads per core
ST = B * S                  # 4096 flattened rows
QHD = QH * HD               # 256 (q hd dims per core)
SCALE = 1.0 / np.sqrt(HD)

BF16 = mybir.dt.bfloat16
F32 = mybir.dt.float32

DC = D // 128               # 16 contraction chunks
SC_N = S // 512             # 4 s-chunks per batch for projections
KB_N = S // 128             # 16 key blocks per batch
KP_N = KB_N // 2            # 8 key-block pairs
QJ_N = S // 512             # 4 q-jobs of 512 per batch


def _rebalance_matmul_waits(nc):
    """walrus allows only one sync-wait on a Matmult. Tile occasionally emits
    two (psum-slot release + engine ordering) on the first matmul of an
    accumulation group. The dedicated Ldweights directly preceding the matmul
    runs on the same in-order PE queue and virtually never carries a wait, so
    shifting the surplus waits onto it preserves ordering semantics."""
    for fn in nc.m.functions:
        for blk in fn.blocks:
            insts = list(blk.instructions)
            for idx, inst in enumerate(insts):
                if type(inst).__name__ != "InstMatmult":
                    continue
                si = inst.sync_info
                waits = list(si.on_wait or []) if si else []
                if len(waits) <= 1:
                    continue
                prev = insts[idx - 1] if idx else None
                assert prev is not None and type(prev).__name__ == "InstLdweights", (
                    f"matmul {inst.name} has {len(waits)} waits but no "
                    f"preceding Ldweights (got {type(prev).__name__})")
                _shift_waits(inst, si, waits, prev)


def _shift_waits(inst, si, waits, carrier):
    psi = carrier.sync_info
    pwaits = list(psi.on_wait or []) if psi else []
    assert len(pwaits) + len(waits) - 1 <= 3, (
        f"{inst.name}: too many combined waits on carrier {carrier.name}")
    moved, kept = waits[:-1], waits[-1:]
    if psi is None:
        carrier.sync_info = type(si)(on_wait=moved, on_update=[])
    else:
        psi.on_wait = pwaits + moved
    si.on_wait = kept


def _rebalance_dma_waits(nc):
    """Same single-wait limit applies to HWDGE DMACopy / gpsimd DMA-direct
    instructions. These always read an SBUF tile written by a producer
    (DVE copy / reciprocal) a few instructions earlier; the producer's
    engine tolerates 3 waits, and since the DMA already waits on the
    producer, conditions moved onto the producer still hold when the DMA
    starts."""
    for fn in nc.m.functions:
        for blk in fn.blocks:
            insts = list(blk.instructions)
            sp_seen = {}   # sem name -> max value already awaited on SP queue
            for idx, inst in enumerate(insts):
                if type(inst).__name__ not in (
                        "InstDMACopy", "InstPartitionBroadcast"):
                    continue
                si = inst.sync_info
                waits = list(si.on_wait or []) if si else []
                is_sp = str(inst.engine) == "EngineType.SP"
                if is_sp and waits:
                    # SP executes serially: waits dominated by an earlier SP
                    # instruction's wait on the same sem are redundant
                    live = [w for w in waits
                            if sp_seen.get(w.ant_name, -1) < w.wait_value]
                    if len(live) < len(waits):
                        si.on_wait = live
                        waits = live
                if is_sp:
                    for w in waits:
                        if sp_seen.get(w.ant_name, -1) < w.wait_value:
                            sp_seen[w.ant_name] = w.wait_value
                if len(waits) <= 1:
                    continue
                src = inst.ins[0].memref if inst.ins else None
                prod = None
                for j in range(idx - 1, max(-1, idx - 400), -1):
                    p = insts[j]
                    pouts = getattr(p, "outs", None)
                    if pouts and pouts[0].memref == src and \
                            type(p).__name__ not in ("InstDMACopy",):
                        prod = p
                        break
                if prod is None:
                    # DRAM load: no producer. SP executes serially, so the
                    # nearest preceding wait-free SP DMA can absorb the
                    # engine-WAR wait; the queue wait stays on this DMA.
                    carrier = None
                    for j in range(idx - 1, max(-1, idx - 400), -1):
                        p = insts[j]
                        if type(p).__name__ == "InstDMACopy" and \
                                str(p.engine) == "EngineType.SP":
                            pw = list(p.sync_info.on_wait or []) \
                                if p.sync_info else []
                            if not pw:
                                carrier = p
                                break
                    if carrier is None:
                        # The engine-WAR wait (kept) implies the slot's
                        # previous DMA write completed (its readers waited on
                        # it), so the same-queue WAW wait is redundant.
                        keep = [w for w in waits if "DMAHW" not in w.ant_name]
                        assert len(keep) == 1, (
                            f"{inst.name}: unexpected pair "
                            f"{[(w.ant_name, w.wait_value) for w in waits]}")
                        si.on_wait = keep
                        continue
                    waits.sort(key=lambda w: 1 if "DMAHW" in w.ant_name else 0)
                    _shift_waits(inst, si, waits, carrier)
                    continue
                # keep the producer-engine wait on the DMA, move the rest
                eng = str(prod.engine)
                key = {"EngineType.DVE": "DVE", "EngineType.ACT": "Activation",
                       "EngineType.Pool": "Pool", "EngineType.PE": "PE",
                       "EngineType.SP": "SP"}.get(eng, "zz")
                waits.sort(key=lambda w: 0 if w.ant_name.startswith(key) else 1)
                waits = waits[::-1]  # producer wait last -> kept
                psi = prod.sync_info
                pn = len(list(psi.on_wait or [])) if psi else 0
                if pn + len(waits) - 1 <= 3:
                    _shift_waits(inst, si, waits, prod)
                else:
                    # producer full: queue wait is FIFO-covered (slot reuse
                    # distance is a multiple of the 8 round-robin queues)
                    keep = [w for w in waits if "DMAHW" not in w.ant_name]
                    assert len(keep) == 1, (
                        f"{inst.name}: unexpected {[(w.ant_name, w.wait_value) for w in waits]}")
                    si.on_wait = keep


def _pin_act_tables(nc):
    """The act-table pass picks the first table containing each activation's
    function, so a kernel using Exp and Ln thrashes between table 0
    (exp_and_others) and table 5 (natural_log, which lacks exp) — one
    1.28us ACT table load per job boundary. All functions this kernel uses
    (Exp, Ln, Copy) live together in 'natural_log_exp_and_others', so pin
    the first load to that table and drop the rest (they carry no waits or
    semaphore updates)."""
    from concourse.hw_specs import get_activation_tables

    tables = get_activation_tables(nc.m.arch)
    nl_id = list(tables).index("natural_log_exp_and_others")
    fns = tables["natural_log_exp_and_others"]
    for need in (mybir.ActivationFunctionType.Exp,
                 mybir.ActivationFunctionType.Ln,
                 mybir.ActivationFunctionType.Copy):
        assert need in fns, need
    for fn in nc.m.functions:
        for blk in fn.blocks:
            first = True
            kept = []
            for inst in blk.instructions:
                if isinstance(inst, mybir.InstLoadActFuncSet):
                    si = inst.sync_info
                    assert not (si and (si.on_wait or si.on_update)), inst.name
                    if not first:
                        continue
                    inst.act_func_set_id = nl_id
                    first = False
                kept.append(inst)
            blk.instructions[:] = kept


def build_program(trace_friendly: bool = False):
    nc = bacc.Bacc("TRN2", target_bir_lowering=False)
    ht = nc.dram_tensor("ht", [D, ST], BF16, kind="ExternalInput")
    wq = nc.dram_tensor("wq", [D, QHD], BF16, kind="ExternalInput")
    wkv = nc.dram_tensor("wkv", [D, 2 * HD], BF16, kind="ExternalInput")
    wo = nc.dram_tensor("wo", [QHD, D], BF16, kind="ExternalInput")
    out = nc.dram_tensor("out", [ST, D], BF16, kind="ExternalOutput")

    with tile.TileContext(nc) as tc:
        with (
            tc.tile_pool(name="singles", bufs=1) as singles,
            tc.tile_pool(name="hstream", bufs=3) as hstream,
            tc.tile_pool(name="expp", bufs=4) as expp,
            tc.tile_pool(name="araw", bufs=2) as arawp,
            tc.tile_pool(name="attn", bufs=2) as attnp,
            tc.tile_pool(name="norm", bufs=2) as normp,
            tc.tile_pool(name="ostage", bufs=4) as ostage,
            tc.tile_pool(name="ps_sc", bufs=2, space="PSUM") as ps_sc,
            tc.tile_pool(name="ps_out", bufs=1, space="PSUM") as ps_out,
            tc.tile_pool(name="ps_op", bufs=2, space="PSUM") as ps_op,
        ):
            # ---- resident weights ----
            # Load order matters for the lead-in: wkv (needed by the first
            # matmul) goes first; the first h chunk is DMA'd right after in
            # proj_phase; wq follows; wo is only needed once the first
            # o_proj group runs (one full job later), so it loads last.
            wq_sb = singles.tile([128, DC, QHD], BF16)
            wkv_sb = singles.tile([128, DC, 2 * HD], BF16)
            wo_sb = singles.tile([128, 2, D], BF16)
            nc.sync.dma_start(
                wkv_sb[:, :, :],
                wkv[:, :].rearrange("(a p) j -> p a j", p=128))

            # identity for the PE V-transposes
            id_sb = singles.tile([64, HD], BF16)
            masks.make_identity(nc, id_sb[:, :])

            # ---- resident activations (per batch) ----
            # qtdup[h][b]: [128, S], Q^T duplicated on both partition halves
            qtdup = [[singles.tile([128, S], BF16, tag=f"qt{h}_{b}",
                                   name=f"qt{h}_{b}")
                      for b in range(B)] for h in range(QH)]
            # kt2[b]: [128, KP_N, 128]; rows 0:64 = even kb K^T, 64:128 = odd
            kt2 = [singles.tile([128, KP_N, 128], BF16, tag=f"kt{b}",
                                name=f"kt{b}") for b in range(B)]
            vaug = [singles.tile([128, KB_N, HD + 1], BF16, tag=f"vaug{b}",
                                 name=f"vaug{b}") for b in range(B)]
            # V^T staging: rows 64:128 written by DVE (in-partition from
            # PSUM), rows 0:64 filled by SBUF->SBUF DMA; transposed at the
            # end of the projection phase.
            vt_sb = [singles.tile([128, S], BF16, tag=f"vt{b}", name=f"vt{b}")
                     for b in range(B)]
            for b in range(B):
                nc.vector.memset(vaug[b][:, :, HD:HD + 1], 1.0)

            # pending o_proj work from the previous attention job: a list of
            # closures, each one (2 matmuls + a DVE cast [+ DMA]).
            pending = []

            def run_pending(n):
                for _ in range(min(n, len(pending))):
                    pending.pop(0)()

            def proj_phase(b, pieces=None, split=False):
                """Emit batch-b projection work as ~1.7us closures.

                pieces=None: everything inline. split=True (with pieces):
                KV + Q-heads-0/1 + V-transposes run inline, Q-heads-2/3 are
                appended to pieces (their h chunks are re-loaded, trading
                ~8.4MB of DMA reads to start attention ~20us earlier).
                split=False with pieces: everything is appended."""
                inline = lambda f: f()
                emit = inline if pieces is None else pieces.append
                h_tiles = {}
                pkv_t = {}
                pq_t = {}

                def load(sc, p2=False, b=b):
                    if sc >= SC_N:
                        return
                    h_sb = hstream.tile([128, DC, 512], BF16, tag="h")
                    h_tiles[(p2, sc)] = h_sb
                    scol = b * S + sc * 512
                    # two halves so the first matmuls start at ~1MB loaded
                    for hf in range(2):
                        nc.sync.dma_start(
                            h_sb[:, 8 * hf:8 * hf + 8, :],
                            ht[1024 * hf:1024 * hf + 1024,
                               scol:scol + 512].rearrange(
                                "(a p) j -> p a j", p=128))

                def kv_a(sc, b=b):
                    load(sc + 1)
                    h_sb = h_tiles[(False, sc)]
                    pkv = ps_op.tile([128, 512], F32, tag="po")
                    pkv_t[sc] = pkv
                    for dc in range(8):
                        nc.tensor.matmul(pkv, wkv_sb[:, dc], h_sb[:, dc],
                                         start=(dc == 0), stop=False)

                def kv_b(sc, b=b):
                    h_sb = h_tiles[(False, sc)]
                    pkv = pkv_t.pop(sc)
                    scol = sc * 512
                    for dc in range(8, DC):
                        nc.tensor.matmul(pkv, wkv_sb[:, dc], h_sb[:, dc],
                                         start=False, stop=(dc == DC - 1))
                    # K^T rows 0:64 -> kt2: kb 4sc+j; even j -> low half
                    # (in-partition), odd j -> high half (cross-partition)
                    for j in range(4):
                        kp, half = (4 * sc + j) // 2, (4 * sc + j) % 2
                        nc.vector.tensor_copy(
                            kt2[b][64 * half:64 * half + 64, kp, :],
                            pkv[0:64, j * 128:(j + 1) * 128])
                    # V^T rows 64:128 -> staging high half, then DMA down
                    nc.vector.tensor_copy(
                        vt_sb[b][64:128, scol:scol + 512], pkv[64:128, :])
                    nc.sync.dma_start(
                        vt_sb[b][0:64, scol:scol + 512],
                        vt_sb[b][64:128, scol:scol + 512])

                def q_a(sc, m, p2, b=b):
                    h_sb = h_tiles[(p2, sc)]
                    pq = ps_op.tile([128, 512], F32, tag="po")
                    pq_t[(sc, m)] = pq
                    for dc in range(8):
                        nc.tensor.matmul(
                            pq, wq_sb[:, dc, m * 128:(m + 1) * 128],
                            h_sb[:, dc], start=(dc == 0), stop=False)

                def q_b(sc, m, p2, b=b):
                    scol = sc * 512
                    h_sb = h_tiles[(p2, sc)]
                    if p2:
                        load(sc + 2, p2=True)
                    pq = pq_t.pop((sc, m))
                    for dc in range(8, DC):
                        nc.tensor.matmul(
                            pq, wq_sb[:, dc, m * 128:(m + 1) * 128],
                            h_sb[:, dc], start=False, stop=(dc == DC - 1))
                    h0, h1 = 2 * m, 2 * m + 1
                    nc.vector.tensor_copy(
                        qtdup[h0][b][0:64, scol:scol + 512], pq[0:64, :])
                    nc.vector.tensor_copy(
                        qtdup[h1][b][64:128, scol:scol + 512], pq[64:128, :])
                    # duplicate onto the other partition half (DMA)
                    nc.sync.dma_start(
                        qtdup[h0][b][64:128, scol:scol + 512],
                        qtdup[h0][b][0:64, scol:scol + 512])
                    nc.sync.dma_start(
                        qtdup[h1][b][0:64, scol:scol + 512],
                        qtdup[h1][b][64:128, scol:scol + 512])

                def vtrans(g, b=b):
                    tr = ps_op.tile([128, 4, HD], BF16, tag="po")
                    for j in range(4):
                        kb = 4 * g + j
                        nc.tensor.transpose(
                            tr[:, j, :],
                            vt_sb[b][0:64, kb * 128:(kb + 1) * 128],
                            id_sb[:, :])
                    nc.vector.tensor_copy(
                        vaug[b][:, 4 * g:4 * g + 4, 0:HD], tr[:, :, :])

                load(0)
                if split:
                    assert pieces is not None
                    for sc in range(SC_N):
                        if sc == SC_N - 1:
                            # fired here (not earlier) so the hstream ring
                            # never makes a pass1 load wait on pass2 readers
                            load(0, p2=True)
                        kv_a(sc), kv_b(sc)
                        q_a(sc, 0, False), q_b(sc, 0, False)
                    for g in range(4):
                        vtrans(g)
                    load(1, p2=True)
                    for sc in range(SC_N):
                        pieces.append(functools.partial(q_a, sc, 1, True))
                        pieces.append(functools.partial(q_b, sc, 1, True))
                else:
                    for sc in range(SC_N):
                        emit(functools.partial(kv_a, sc))
                        emit(functools.partial(kv_b, sc))
                        for m in range(2):
                            emit(functools.partial(q_a, sc, m, False))
                            emit(functools.partial(q_b, sc, m, False))
                    for g in range(4):
                        emit(functools.partial(vtrans, g))

            # ---------- flat software-pipelined attention ----------
            pieces = []
            cur = {"outp": None, "araw": None, "expT": {}}

            def emit_scores(s):
                b, qj, h, kp = s
                q0 = qj * 512
                scp = ps_sc.tile([128, 1024], F32)
                nc.tensor.matmul(
                    scp[:, 0:512], kt2[b][0:64, kp, :],
                    qtdup[h][b][0:64, q0:q0 + 512], start=True, stop=True)
                nc.tensor.matmul(
                    scp[:, 512:1024], kt2[b][64:128, kp, :],
                    qtdup[h][b][64:128, q0:q0 + 512], start=True, stop=True)
                expT = expp.tile([128, 1024], BF16)
                nc.scalar.activation(expT[:, :], scp[:, :],
                                     mybir.ActivationFunctionType.Exp,
                                     scale=SCALE)
                cur["expT"][s] = expT

            def finish_head(b, h, araw, attn_sb):
                # per-head normalization, spread through the job so no ACT
                # lump ever stalls the exp stream: 1/den as exp(-ln(den)).
                lnv = normp.tile([1, 512], F32, tag="lnv")
                nc.scalar.activation(lnv, araw[64:65, h, :],
                                     mybir.ActivationFunctionType.Ln)
                recip = normp.tile([1, 512], BF16, tag="recip")
                nc.scalar.activation(recip, lnv,
                                     mybir.ActivationFunctionType.Exp,
                                     scale=-1.0)
                bcast = normp.tile([64, 512], BF16, tag="bcast")
                nc.gpsimd.partition_broadcast(bcast, recip)
                nc.vector.tensor_mul(
                    attn_sb[(h % 2) * 64:(h % 2) * 64 + 64, h // 2, :],
                    araw[0:64, h, :], bcast[:, :])

            def queue_oproj(b, qj, attn_sb, last):
                q0 = qj * 512
                for qc in range(4):
                    ost = ostage.tile([128, 2048], BF16)
                    row = b * S + q0 + qc * 128
                    for nb in range(4):
                        def grp(qc=qc, nb=nb, ost=ost, row=row,
                                attn_sb=attn_sb, act_cast=(last and nb % 2)):
                            po = ps_op.tile([128, 512], F32, tag="po")
                            for hh in range(2):
                                nc.tensor.matmul(
                                    po,
                                    attn_sb[:, hh, qc * 128:(qc + 1) * 128],
                                    wo_sb[:, hh, nb * 512:(nb + 1) * 512],
                                    start=(hh == 0), stop=(hh == 1))
                            if act_cast:
                                nc.scalar.copy(
                                    ost[:, nb * 512:(nb + 1) * 512], po)
                            else:
                                nc.vector.tensor_copy(
                                    ost[:, nb * 512:(nb + 1) * 512], po)
                            if nb == 3:
                                for dd in range(2):
                                    nc.sync.dma_start(
                                        out[row:row + 128,
                                            dd * 1024:(dd + 1) * 1024],
                                        ost[:, dd * 1024:(dd + 1) * 1024])
                        pending.append(grp)

            def emit_pv(s, last):
                b, qj, h, kp = s
                expT = cur["expT"].pop(s)
                if kp == 0:
                    cur["outp"] = ps_out.tile([HD + 1, 2, 512], F32,
                                              name="pvab", tag="pvab")
                    if h == 0:
                        cur["araw"] = arawp.tile([65, QH, 512], BF16, name="araw", tag="araw")
                outp = cur["outp"]
                # row-tiled PV: key-halves of each kb run concurrently in
                # the PE array (tile rows 0-63 / 64-127), accumulating into
                # two separate banks; the drain adds them (and the two
                # denominator halves) back together.
                for kb2 in (2 * kp, 2 * kp + 1):
                    ex = expT[:, 0:512] if kb2 == 2 * kp else expT[:, 512:1024]
                    nc.tensor.matmul(outp[:, 0, :], vaug[b][0:64, kb2, :],
                                     ex[0:64, :],
                                     start=(kp == 0 and kb2 == 2 * kp),
                                     stop=(kp == KP_N - 1 and kb2 != 2 * kp))
                    nc.tensor.matmul(outp[:, 1, :], vaug[b][64:128, kb2, :],
                                     ex[64:128, :],
                                     start=(kp == 0 and kb2 == 2 * kp),
                                     stop=(kp == KP_N - 1 and kb2 != 2 * kp))
                if kp == KP_N - 1:
                    # drain this head's accumulator (frees PSUM fast).
                    # DVE may read only one PSUM operand per instruction, so
                    # copy bank A out then add bank B in place.
                    nc.vector.tensor_copy(cur["araw"][:, h, :], outp[:, 0, :])
                    nc.vector.tensor_add(cur["araw"][:, h, :],
                                         cur["araw"][:, h, :], outp[:, 1, :])
                    if h == 0:
                        cur["attn"] = attnp.tile([128, 2, 512], BF16,
                                                 name="attn_sb", tag="attn")
                    finish_head(b, h, cur["araw"], cur["attn"])
                    if h == QH - 1:
                        queue_oproj(b, qj, cur["attn"], last)
                        if last:
                            run_pending(len(pending))
                if kp in (2, 4, 7) and pieces:
                    pieces.pop(0)()
                if (h == 0 and kp >= 4) or (h > 0 and kp % 2 == 1):
                    run_pending(1)

            # ================= schedule =================
            # wq rides behind wkv + the first h chunk; wo (first needed by
            # o_proj of job (0,0), a full job later) loads after proj-b0.
            nc.sync.dma_start(
                wq_sb[:, :, :], wq[:, :].rearrange("(a p) j -> p a j", p=128))
            proj_phase(0, pieces=pieces, split=True)
            nc.sync.dma_start(
                wo_sb[:, :, :], wo[:, :].rearrange("(a p) j -> p a j", p=128))
            proj_phase(1, pieces=pieces)

            slots = [(b, qj, h, kp)
                     for b in range(B) for qj in range(QJ_N)
                     for h in range(QH) for kp in range(KP_N)]
            last_slot = slots[-1]
            prev = None
            for s in slots:
                if s[:3] == (1, 0, 0) and s[3] == 0:
                    assert not pieces, len(pieces)
                emit_scores(s)
                if prev is not None:
                    emit_pv(prev, last=False)
                prev = s
            emit_pv(prev, last=True)
            run_pending(len(pending))
    nc.compile()
    _pin_act_tables(nc)
    _rebalance_matmul_waits(nc)
    _rebalance_dma_waits(nc)
    return nc


@functools.lru_cache(maxsize=1)
def _get_program():
    return build_program()


def _in_maps(hidden_states, Wq, Wk, Wv, Wo):
    bf = ml_dtypes.bfloat16
    htT = np.ascontiguousarray(
        hidden_states.reshape(ST, D).T.astype(bf))          # [D, B*S]
    in_maps = []
    for c in range(NCORES):
        wkv = np.concatenate(
            [Wk[:, c * HD:(c + 1) * HD], Wv[:, c * HD:(c + 1) * HD]], axis=1)
        in_maps.append({
            "ht": htT,
            "wq": np.ascontiguousarray(Wq[:, c * QHD:(c + 1) * QHD].astype(bf)),
            "wkv": np.ascontiguousarray(wkv.astype(bf)),
            "wo": np.ascontiguousarray(Wo[c * QHD:(c + 1) * QHD, :].astype(bf)),
        })
    return in_maps


def kernel(hidden_states, Wq, Wk, Wv, Wo):
    hidden_states = np.asarray(hidden_states)
    Wq, Wk, Wv, Wo = (np.asarray(x) for x in (Wq, Wk, Wv, Wo))
    in_maps = _in_maps(hidden_states, Wq, Wk, Wv, Wo)
    nc = _get_program()
    res = run_bass_kernel_spmd(nc, in_maps, core_ids=list(range(NCORES)))
    total = res.results[0]["out"].astype(np.float64)
    for c in range(1, NCORES):
        total += res.results[c]["out"].astype(np.float64)
    return total.reshape(B, S, D).astype(np.float32)


# revision 21
# speedup vs baseline: 1.1375x; 1.0658x over previous
"""GQA attention (B=2, S=2048, D=2048, 32 Q heads / 8 KV heads, HD=64) on 8 trn2 cores.

Sharding: tensor-parallel over heads. Core c gets Q heads [4c, 4c+4), KV head c.
Each core computes a full [B*S, D] partial of the output (its 4 heads through
o_proj); the host sums the 8 partials. No collectives.

v2 design (vs the v1 baseline):
  - K and V projections merged into one matmul stream (stationary [wk|wv]
    [128,128]) -> K^T rows 0-63, V^T rows 64-127 of each PSUM tile. V^T is
    turned into natural V via PE transposes (the v1 h-stationary V projection
    was Ldweights-bound).
  - Scores matmuls are row-tiled pairs: contraction is HD=64, so two key
    blocks' K^T stationaries sit on partition halves (kt2[0:64]=even kb,
    kt2[64:128]=odd kb) and the two matmuls run CONCURRENTLY in the PE array
    (tile_position row groups, auto-derived from base partitions). Q is
    duplicated onto both partition halves via SBUF->SBUF DMA.
  - Normalization is job-level and decoupled: PV accumulators (PSUM) are
    drained per head to SBUF by one DVE copy, then recip -> gpsimd
    partition_broadcast -> DVE muls happen off the critical PE path
    (outp bufs=2 so the next head's PV never waits).
  - o_proj of job J is interleaved into job J+1's kb loop (and into the
    batch-1 projection phase) so the PE never idles while ACT does exp.
  - Output is written bf16 (host accumulates partials in f64).
"""

import functools

import numpy as np
import ml_dtypes

import concourse.bacc as bacc
import concourse.bass as bass
import concourse.mybir as mybir
import concourse.tile as tile
from concourse import masks
from concourse.bass_utils import run_bass_kernel_spmd

B, S, D = 2, 2048, 2048
H, KVH, HD = 32, 8, 64
NCORES = 8
QH = H // NCORES            # 4 q heads per core
ST = B * S                  # 4096 flattened rows
QHD = QH * HD               # 256 (q hd dims per core)
SCALE = 1.0 / np.sqrt(HD)

BF16 = mybir.dt.bfloat16
F32 = mybir.dt.float32

DC = D // 128               # 16 contraction chunks
SC_N = S // 512             # 4 s-chunks per batch for projections
KB_N = S // 128             # 16 key blocks per batch
KP_N = KB_N // 2            # 8 key-block pairs
QJ_N = S // 512             # 4 q-jobs of 512 per batch


def _rebalance_matmul_waits(nc):
    """walrus allows only one sync-wait on a Matmult. Tile occasionally emits
    two (psum-slot release + engine ordering) on the first matmul of an
    accumulation group. The dedicated Ldweights directly preceding the matmul
    runs on the same in-order PE queue and virtually never carries a wait, so
    shifting the surplus waits onto it preserves ordering semantics."""
    for fn in nc.m.functions:
        for blk in fn.blocks:
            insts = list(blk.instructions)
            for idx, inst in enumerate(insts):
                if type(inst).__name__ != "InstMatmult":
                    continue
                si = inst.sync_info
                waits = list(si.on_wait or []) if si else []
                if len(waits) <= 1:
                    continue
                prev = insts[idx - 1] if idx else None
                assert prev is not None and type(prev).__name__ == "InstLdweights", (
                    f"matmul {inst.name} has {len(waits)} waits but no "
                    f"preceding Ldweights (got {type(prev).__name__})")
                _shift_waits(inst, si, waits, prev)


def _shift_waits(inst, si, waits, carrier):
    psi = carrier.sync_info
    pwaits = list(psi.on_wait or []) if psi else []
    assert len(pwaits) + len(waits) - 1 <= 3, (
        f"{inst.name}: too many combined waits on carrier {carrier.name}")
    moved, kept = waits[:-1], waits[-1:]
    if psi is None:
        carrier.sync_info = type(si)(on_wait=moved, on_update=[])
    else:
        psi.on_wait = pwaits + moved
    si.on_wait = kept


def _rebalance_dma_waits(nc):
    """Same single-wait limit applies to HWDGE DMACopy / gpsimd DMA-direct
    instructions. These always read an SBUF tile written by a producer
    (DVE copy / reciprocal) a few instructions earlier; the producer's
    engine tolerates 3 waits, and since the DMA already waits on the
    producer, conditions moved onto the producer still hold when the DMA
    starts."""
    for fn in nc.m.functions:
        for blk in fn.blocks:
            insts = list(blk.instructions)
            sp_seen = {}   # sem name -> max value already awaited on SP queue
            for idx, inst in enumerate(insts):
                if type(inst).__name__ not in (
                        "InstDMACopy", "InstPartitionBroadcast"):
                    continue
                si = inst.sync_info
                waits = list(si.on_wait or []) if si else []
                is_sp = str(inst.engine) == "EngineType.SP"
                if is_sp and waits:
                    # SP executes serially: waits dominated by an earlier SP
                    # instruction's wait on the same sem are redundant
                    live = [w for w in waits
                            if sp_seen.get(w.ant_name, -1) < w.wait_value]
                    if len(live) < len(waits):
                        si.on_wait = live
                        waits = live
                if is_sp:
                    for w in waits:
                        if sp_seen.get(w.ant_name, -1) < w.wait_value:
                            sp_seen[w.ant_name] = w.wait_value
                if len(waits) <= 1:
                    continue
                src = inst.ins[0].memref if inst.ins else None
                prod = None
                for j in range(idx - 1, max(-1, idx - 400), -1):
                    p = insts[j]
                    pouts = getattr(p, "outs", None)
                    if pouts and pouts[0].memref == src and \
                            type(p).__name__ not in ("InstDMACopy",):
                        prod = p
                        break
                if prod is None:
                    # DRAM load: no producer. SP executes serially, so the
                    # nearest preceding wait-free SP DMA can absorb the
                    # engine-WAR wait; the queue wait stays on this DMA.
                    carrier = None
                    for j in range(idx - 1, max(-1, idx - 400), -1):
                        p = insts[j]
                        if type(p).__name__ == "InstDMACopy" and \
                                str(p.engine) == "EngineType.SP":
                            pw = list(p.sync_info.on_wait or []) \
                                if p.sync_info else []
                            if not pw:
                                carrier = p
                                break
                    if carrier is None:
                        # The engine-WAR wait (kept) implies the slot's
                        # previous DMA write completed (its readers waited on
                        # it), so the same-queue WAW wait is redundant.
                        keep = [w for w in waits if "DMAHW" not in w.ant_name]
                        assert len(keep) == 1, (
                            f"{inst.name}: unexpected pair "
                            f"{[(w.ant_name, w.wait_value) for w in waits]}")
                        si.on_wait = keep
                        continue
                    waits.sort(key=lambda w: 1 if "DMAHW" in w.ant_name else 0)
                    _shift_waits(inst, si, waits, carrier)
                    continue
                # keep the producer-engine wait on the DMA, move the rest
                eng = str(prod.engine)
                key = {"EngineType.DVE": "DVE", "EngineType.ACT": "Activation",
                       "EngineType.Pool": "Pool", "EngineType.PE": "PE",
                       "EngineType.SP": "SP"}.get(eng, "zz")
                waits.sort(key=lambda w: 0 if w.ant_name.startswith(key) else 1)
                waits = waits[::-1]  # producer wait last -> kept
                psi = prod.sync_info
                pn = len(list(psi.on_wait or [])) if psi else 0
                if pn + len(waits) - 1 <= 3:
                    _shift_waits(inst, si, waits, prod)
                else:
                    # producer full: queue wait is FIFO-covered (slot reuse
                    # distance is a multiple of the 8 round-robin queues)
                    keep = [w for w in waits if "DMAHW" not in w.ant_name]
                    assert len(keep) == 1, (
                        f"{inst.name}: unexpected {[(w.ant_name, w.wait_value) for w in waits]}")
                    si.on_wait = keep


def _pin_act_tables(nc):
    """The act-table pass picks the first table containing each activation's
    function, so a kernel using Exp and Ln thrashes between table 0
    (exp_and_others) and table 5 (natural_log, which lacks exp) — one
    1.28us ACT table load per job boundary. All functions this kernel uses
    (Exp, Ln, Copy) live together in 'natural_log_exp_and_others', so pin
    the first load to that table and drop the rest (they carry no waits or
    semaphore updates)."""
    from concourse.hw_specs import get_activation_tables

    tables = get_activation_tables(nc.m.arch)
    nl_id = list(tables).index("natural_log_exp_and_others")
    fns = tables["natural_log_exp_and_others"]
    for need in (mybir.ActivationFunctionType.Exp,
                 mybir.ActivationFunctionType.Ln,
                 mybir.ActivationFunctionType.Copy):
        assert need in fns, need
    for fn in nc.m.functions:
        for blk in fn.blocks:
            first = True
            kept = []
            for inst in blk.instructions:
                if isinstance(inst, mybir.InstLoadActFuncSet):
                    si = inst.sync_info
                    assert not (si and (si.on_wait or si.on_update)), inst.name
                    if not first:
                        continue
                    inst.act_func_set_id = nl_id
                    first = False
                kept.append(inst)
            blk.instructions[:] = kept


def build_program(trace_friendly: bool = False):
    nc = bacc.Bacc("TRN2", target_bir_lowering=False)
    ht = nc.dram_tensor("ht", [D, ST], BF16, kind="ExternalInput")
    wq = nc.dram_tensor("wq", [D, QHD], BF16, kind="ExternalInput")
    wkv = nc.dram_tensor("wkv", [D, 2 * HD], BF16, kind="ExternalInput")
    wo = nc.dram_tensor("wo", [QHD, D], BF16, kind="ExternalInput")
    out = nc.dram_tensor("out", [ST, D], BF16, kind="ExternalOutput")

    with tile.TileContext(nc) as tc:
        with (
            tc.tile_pool(name="singles", bufs=1) as singles,
            tc.tile_pool(name="hstream", bufs=3) as hstream,
            tc.tile_pool(name="expp", bufs=4) as expp,
            tc.tile_pool(name="araw", bufs=2) as arawp,
            tc.tile_pool(name="attn", bufs=2) as attnp,
            tc.tile_pool(name="norm", bufs=2) as normp,
            tc.tile_pool(name="ostage", bufs=4) as ostage,
            tc.tile_pool(name="ps_sc", bufs=2, space="PSUM") as ps_sc,
            tc.tile_pool(name="ps_out", bufs=2, space="PSUM") as ps_out,
            tc.tile_pool(name="ps_op", bufs=2, space="PSUM") as ps_op,
        ):
            # ---- resident weights ----
            # Load order matters for the lead-in: wkv (needed by the first
            # matmul) goes first; the first h chunk is DMA'd right after in
            # proj_phase; wq follows; wo is only needed once the first
            # o_proj group runs (one full job later), so it loads last.
            wq_sb = singles.tile([128, DC, QHD], BF16)
            wkv_sb = singles.tile([128, DC, 2 * HD], BF16)
            wo_sb = singles.tile([128, 2, D], BF16)
            nc.sync.dma_start(
                wkv_sb[:, :, :],
                wkv[:, :].rearrange("(a p) j -> p a j", p=128))

            # identity for the PE V-transposes
            id_sb = singles.tile([64, HD], BF16)
            masks.make_identity(nc, id_sb[:, :])

            # ---- resident activations (per batch) ----
            # qtdup[h][b]: [128, S], Q^T duplicated on both partition halves
            qtdup = [[singles.tile([128, S], BF16, tag=f"qt{h}_{b}",
                                   name=f"qt{h}_{b}")
                      for b in range(B)] for h in range(QH)]
            # kt2[b]: [128, KP_N, 128]; rows 0:64 = even kb K^T, 64:128 = odd
            kt2 = [singles.tile([128, KP_N, 128], BF16, tag=f"kt{b}",
                                name=f"kt{b}") for b in range(B)]
            vaug = [singles.tile([128, KB_N, HD + 1], BF16, tag=f"vaug{b}",
                                 name=f"vaug{b}") for b in range(B)]
            # V^T staging: rows 64:128 written by DVE (in-partition from
            # PSUM), rows 0:64 filled by SBUF->SBUF DMA; transposed at the
            # end of the projection phase.
            vt_sb = [singles.tile([128, S], BF16, tag=f"vt{b}", name=f"vt{b}")
                     for b in range(B)]
            for b in range(B):
                nc.vector.memset(vaug[b][:, :, HD:HD + 1], 1.0)

            # pending o_proj work from the previous attention job: a list of
            # closures, each one (2 matmuls + a DVE cast [+ DMA]).
            pending = []

            def run_pending(n):
                for _ in range(min(n, len(pending))):
                    pending.pop(0)()

            def proj_phase(b, pieces=None, split=False):
                """Emit batch-b projection work as ~1.7us closures.

                pieces=None: everything inline. split=True (with pieces):
                KV + Q-heads-0/1 + V-transposes run inline, Q-heads-2/3 are
                appended to pieces (their h chunks are re-loaded, trading
                ~8.4MB of DMA reads to start attention ~20us earlier).
                split=False with pieces: everything is appended."""
                inline = lambda f: f()
                emit = inline if pieces is None else pieces.append
                h_tiles = {}
                pkv_t = {}
                pq_t = {}

                def load(sc, p2=False, b=b):
                    if sc >= SC_N:
                        return
                    h_sb = hstream.tile([128, DC, 512], BF16, tag="h")
                    h_tiles[(p2, sc)] = h_sb
                    scol = b * S + sc * 512
                    # two halves so the first matmuls start at ~1MB loaded
                    for hf in range(2):
                        nc.sync.dma_start(
                            h_sb[:, 8 * hf:8 * hf + 8, :],
                            ht[1024 * hf:1024 * hf + 1024,
                               scol:scol + 512].rearrange(
                                "(a p) j -> p a j", p=128))

                def kv_a(sc, b=b):
                    load(sc + 1)
                    h_sb = h_tiles[(False, sc)]
                    pkv = ps_op.tile([128, 512], F32, tag="po")
                    pkv_t[sc] = pkv
                    for dc in range(8):
                        nc.tensor.matmul(pkv, wkv_sb[:, dc], h_sb[:, dc],
                                         start=(dc == 0), stop=False)

                def kv_b(sc, b=b):
                    h_sb = h_tiles[(False, sc)]
                    pkv = pkv_t.pop(sc)
                    scol = sc * 512
                    for dc in range(8, DC):
                        nc.tensor.matmul(pkv, wkv_sb[:, dc], h_sb[:, dc],
                                         start=False, stop=(dc == DC - 1))
                    # K^T rows 0:64 -> kt2: kb 4sc+j; even j -> low half
                    # (in-partition), odd j -> high half (cross-partition)
                    for j in range(4):
                        kp, half = (4 * sc + j) // 2, (4 * sc + j) % 2
                        nc.vector.tensor_copy(
                            kt2[b][64 * half:64 * half + 64, kp, :],
                            pkv[0:64, j * 128:(j + 1) * 128])
                    # V^T rows 64:128 -> staging high half, then DMA down
                    nc.vector.tensor_copy(
                        vt_sb[b][64:128, scol:scol + 512], pkv[64:128, :])
                    nc.sync.dma_start(
                        vt_sb[b][0:64, scol:scol + 512],
                        vt_sb[b][64:128, scol:scol + 512])

                def q_a(sc, m, p2, b=b):
                    h_sb = h_tiles[(p2, sc)]
                    pq = ps_op.tile([128, 512], F32, tag="po")
                    pq_t[(sc, m)] = pq
                    for dc in range(8):
                        nc.tensor.matmul(
                            pq, wq_sb[:, dc, m * 128:(m + 1) * 128],
                            h_sb[:, dc], start=(dc == 0), stop=False)

                def q_b(sc, m, p2, b=b):
                    scol = sc * 512
                    h_sb = h_tiles[(p2, sc)]
                    if p2:
                        load(sc + 2, p2=True)
                    pq = pq_t.pop((sc, m))
                    for dc in range(8, DC):
                        nc.tensor.matmul(
                            pq, wq_sb[:, dc, m * 128:(m + 1) * 128],
                            h_sb[:, dc], start=False, stop=(dc == DC - 1))
                    h0, h1 = 2 * m, 2 * m + 1
                    nc.vector.tensor_copy(
                        qtdup[h0][b][0:64, scol:scol + 512], pq[0:64, :])
                    nc.vector.tensor_copy(
                        qtdup[h1][b][64:128, scol:scol + 512], pq[64:128, :])
                    # duplicate onto the other partition half (DMA)
                    nc.sync.dma_start(
                        qtdup[h0][b][64:128, scol:scol + 512],
                        qtdup[h0][b][0:64, scol:scol + 512])
                    nc.sync.dma_start(
                        qtdup[h1][b][0:64, scol:scol + 512],
                        qtdup[h1][b][64:128, scol:scol + 512])

                def vtrans(g, b=b):
                    tr = ps_op.tile([128, 4, HD], BF16, tag="po")
                    for j in range(4):
                        kb = 4 * g + j
                        nc.tensor.transpose(
                            tr[:, j, :],
                            vt_sb[b][0:64, kb * 128:(kb + 1) * 128],
                            id_sb[:, :])
                    nc.vector.tensor_copy(
                        vaug[b][:, 4 * g:4 * g + 4, 0:HD], tr[:, :, :])

                load(0)
                if split:
                    assert pieces is not None
                    for sc in range(SC_N):
                        if sc == SC_N - 1:
                            # fired here (not earlier) so the hstream ring
                            # never makes a pass1 load wait on pass2 readers
                            load(0, p2=True)
                        kv_a(sc), kv_b(sc)
                        q_a(sc, 0, False), q_b(sc, 0, False)
                    for g in range(4):
                        vtrans(g)
                    load(1, p2=True)
                    for sc in range(SC_N):
                        pieces.append(functools.partial(q_a, sc, 1, True))
                        pieces.append(functools.partial(q_b, sc, 1, True))
                else:
                    for sc in range(SC_N):
                        emit(functools.partial(kv_a, sc))
                        emit(functools.partial(kv_b, sc))
                        for m in range(2):
                            emit(functools.partial(q_a, sc, m, False))
                            emit(functools.partial(q_b, sc, m, False))
                    for g in range(4):
                        emit(functools.partial(vtrans, g))

            # ---------- flat software-pipelined attention ----------
            pieces = []
            cur = {"outp": None, "araw": None, "expT": {}}

            def emit_scores(s):
                b, qj, h, kp = s
                q0 = qj * 512
                scp = ps_sc.tile([128, 1024], F32)
                nc.tensor.matmul(
                    scp[:, 0:512], kt2[b][0:64, kp, :],
                    qtdup[h][b][0:64, q0:q0 + 512], start=True, stop=True)
                nc.tensor.matmul(
                    scp[:, 512:1024], kt2[b][64:128, kp, :],
                    qtdup[h][b][64:128, q0:q0 + 512], start=True, stop=True)
                expT = expp.tile([128, 1024], BF16)
                nc.scalar.activation(expT[:, :], scp[:, :],
                                     mybir.ActivationFunctionType.Exp,
                                     scale=SCALE)
                cur["expT"][s] = expT

            def finish_head(b, h, araw, attn_sb):
                # per-head normalization, spread through the job so no ACT
                # lump ever stalls the exp stream: 1/den as exp(-ln(den)).
                lnv = normp.tile([1, 512], F32, tag="lnv")
                nc.scalar.activation(lnv, araw[64:65, h, :],
                                     mybir.ActivationFunctionType.Ln)
                recip = normp.tile([1, 512], BF16, tag="recip")
                nc.scalar.activation(recip, lnv,
                                     mybir.ActivationFunctionType.Exp,
                                     scale=-1.0)
                bcast = normp.tile([64, 512], BF16, tag="bcast")
                nc.gpsimd.partition_broadcast(bcast, recip)
                nc.vector.tensor_mul(
                    attn_sb[(h % 2) * 64:(h % 2) * 64 + 64, h // 2, :],
                    araw[0:64, h, :], bcast[:, :])

            def queue_oproj(b, qj, attn_sb, last):
                q0 = qj * 512
                for qc in range(4):
                    ost = ostage.tile([128, 2048], BF16)
                    row = b * S + q0 + qc * 128
                    for nb in range(4):
                        def grp(qc=qc, nb=nb, ost=ost, row=row,
                                attn_sb=attn_sb, act_cast=(last and nb % 2)):
                            po = ps_op.tile([128, 512], F32, tag="po")
                            for hh in range(2):
                                nc.tensor.matmul(
                                    po,
                                    attn_sb[:, hh, qc * 128:(qc + 1) * 128],
                                    wo_sb[:, hh, nb * 512:(nb + 1) * 512],
                                    start=(hh == 0), stop=(hh == 1))
                            if act_cast:
                                nc.scalar.copy(
                                    ost[:, nb * 512:(nb + 1) * 512], po)
                            else:
                                nc.vector.tensor_copy(
                                    ost[:, nb * 512:(nb + 1) * 512], po)
                            if nb == 3:
                                for dd in range(2):
                                    nc.sync.dma_start(
                                        out[row:row + 128,
                                            dd * 1024:(dd + 1) * 1024],
                                        ost[:, dd * 1024:(dd + 1) * 1024])
                        pending.append(grp)

            def emit_pv(s, last):
                b, qj, h, kp = s
                expT = cur["expT"].pop(s)
                if kp == 0:
                    cur["outp"] = ps_out.tile([HD + 1, 512], F32,
                                              name="outp", tag="outp")
                    if h == 0:
                        cur["araw"] = arawp.tile([65, QH, 512], BF16, name="araw", tag="araw")
                outp = cur["outp"]
                nc.tensor.matmul(outp, vaug[b][:, 2 * kp, :], expT[:, 0:512],
                                 start=(kp == 0), stop=False)
                nc.tensor.matmul(outp, vaug[b][:, 2 * kp + 1, :],
                                 expT[:, 512:1024],
                                 start=False, stop=(kp == KP_N - 1))
                if kp == KP_N - 1:
                    # drain this head's accumulator (frees PSUM fast)
                    nc.vector.tensor_copy(cur["araw"][:, h, :], outp)
                    if h == 0:
                        cur["attn"] = attnp.tile([128, 2, 512], BF16,
                                                 name="attn_sb", tag="attn")
                    finish_head(b, h, cur["araw"], cur["attn"])
                    if h == QH - 1:
                        queue_oproj(b, qj, cur["attn"], last)
                        if last:
                            run_pending(len(pending))
                if kp in (2, 4, 7) and pieces:
                    pieces.pop(0)()
                if (h == 0 and kp >= 4) or (h > 0 and kp % 2 == 1):
                    run_pending(1)

            # ================= schedule =================
            # wq rides behind wkv + the first h chunk; wo (first needed by
            # o_proj of job (0,0), a full job later) loads after proj-b0.
            nc.sync.dma_start(
                wq_sb[:, :, :], wq[:, :].rearrange("(a p) j -> p a j", p=128))
            proj_phase(0, pieces=pieces, split=True)
            nc.sync.dma_start(
                wo_sb[:, :, :], wo[:, :].rearrange("(a p) j -> p a j", p=128))
            proj_phase(1, pieces=pieces)

            slots = [(b, qj, h, kp)
                     for b in range(B) for qj in range(QJ_N)
                     for h in range(QH) for kp in range(KP_N)]
            last_slot = slots[-1]
            prev = None
            for s in slots:
                if s[:3] == (1, 0, 0) and s[3] == 0:
                    assert not pieces, len(pieces)
                emit_scores(s)
                if prev is not None:
                    emit_pv(prev, last=False)
                prev = s
            emit_pv(prev, last=True)
            run_pending(len(pending))
    nc.compile()
    _pin_act_tables(nc)
    _rebalance_matmul_waits(nc)
    _rebalance_dma_waits(nc)
    return nc


@functools.lru_cache(maxsize=1)
def _get_program():
    return build_program()


def _in_maps(hidden_states, Wq, Wk, Wv, Wo):
    bf = ml_dtypes.bfloat16
    htT = np.ascontiguousarray(
        hidden_states.reshape(ST, D).T.astype(bf))          # [D, B*S]
    in_maps = []
    for c in range(NCORES):
        wkv = np.concatenate(
            [Wk[:, c * HD:(c + 1) * HD], Wv[:, c * HD:(c + 1) * HD]], axis=1)
        in_maps.append({
            "ht": htT,
            "wq": np.ascontiguousarray(Wq[:, c * QHD:(c + 1) * QHD].astype(bf)),
            "wkv": np.ascontiguousarray(wkv.astype(bf)),
            "wo": np.ascontiguousarray(Wo[c * QHD:(c + 1) * QHD, :].astype(bf)),
        })
    return in_maps


def kernel(hidden_states, Wq, Wk, Wv, Wo):
    hidden_states = np.asarray(hidden_states)
    Wq, Wk, Wv, Wo = (np.asarray(x) for x in (Wq, Wk, Wv, Wo))
    in_maps = _in_maps(hidden_states, Wq, Wk, Wv, Wo)
    nc = _get_program()
    res = run_bass_kernel_spmd(nc, in_maps, core_ids=list(range(NCORES)))
    total = res.results[0]["out"].astype(np.float64)
    for c in range(1, NCORES):
        total += res.results[c]["out"].astype(np.float64)
    return total.reshape(B, S, D).astype(np.float32)


# revision 23
# speedup vs baseline: 1.1465x; 1.0079x over previous
"""GQA attention (B=2, S=2048, D=2048, 32 Q heads / 8 KV heads, HD=64) on 8 trn2 cores.

Sharding: tensor-parallel over heads. Core c gets Q heads [4c, 4c+4), KV head c.
Each core computes a full [B*S, D] partial of the output (its 4 heads through
o_proj); the host sums the 8 partials. No collectives.

v2 design (vs the v1 baseline):
  - K and V projections merged into one matmul stream (stationary [wk|wv]
    [128,128]) -> K^T rows 0-63, V^T rows 64-127 of each PSUM tile. V^T is
    turned into natural V via PE transposes (the v1 h-stationary V projection
    was Ldweights-bound).
  - Scores matmuls are row-tiled pairs: contraction is HD=64, so two key
    blocks' K^T stationaries sit on partition halves (kt2[0:64]=even kb,
    kt2[64:128]=odd kb) and the two matmuls run CONCURRENTLY in the PE array
    (tile_position row groups, auto-derived from base partitions). Q is
    duplicated onto both partition halves via SBUF->SBUF DMA.
  - Normalization is job-level and decoupled: PV accumulators (PSUM) are
    drained per head to SBUF by one DVE copy, then recip -> gpsimd
    partition_broadcast -> DVE muls happen off the critical PE path
    (outp bufs=2 so the next head's PV never waits).
  - o_proj of job J is interleaved into job J+1's kb loop (and into the
    batch-1 projection phase) so the PE never idles while ACT does exp.
  - Output is written bf16 (host accumulates partials in f64).
"""

import functools

import numpy as np
import ml_dtypes

import concourse.bacc as bacc
import concourse.bass as bass
import concourse.mybir as mybir
import concourse.tile as tile
from concourse import masks
from concourse.bass_utils import run_bass_kernel_spmd

B, S, D = 2, 2048, 2048
H, KVH, HD = 32, 8, 64
NCORES = 8
QH = H // NCORES            # 4 q heads per core
ST = B * S                  # 4096 flattened rows
QHD = QH * HD               # 256 (q hd dims per core)
SCALE = 1.0 / np.sqrt(HD)

BF16 = mybir.dt.bfloat16
F32 = mybir.dt.float32

DC = D // 128               # 16 contraction chunks
SC_N = S // 512             # 4 s-chunks per batch for projections
KB_N = S // 128             # 16 key blocks per batch
KP_N = KB_N // 2            # 8 key-block pairs
QJ_N = S // 512             # 4 q-jobs of 512 per batch


def _rebalance_matmul_waits(nc):
    """walrus allows only one sync-wait on a Matmult. Tile occasionally emits
    two (psum-slot release + engine ordering) on the first matmul of an
    accumulation group. The dedicated Ldweights directly preceding the matmul
    runs on the same in-order PE queue and virtually never carries a wait, so
    shifting the surplus waits onto it preserves ordering semantics."""
    for fn in nc.m.functions:
        for blk in fn.blocks:
            insts = list(blk.instructions)
            for idx, inst in enumerate(insts):
                if type(inst).__name__ != "InstMatmult":
                    continue
                si = inst.sync_info
                waits = list(si.on_wait or []) if si else []
                if len(waits) <= 1:
                    continue
                prev = insts[idx - 1] if idx else None
                assert prev is not None and type(prev).__name__ == "InstLdweights", (
                    f"matmul {inst.name} has {len(waits)} waits but no "
                    f"preceding Ldweights (got {type(prev).__name__})")
                _shift_waits(inst, si, waits, prev)


def _shift_waits(inst, si, waits, carrier):
    psi = carrier.sync_info
    pwaits = list(psi.on_wait or []) if psi else []
    assert len(pwaits) + len(waits) - 1 <= 3, (
        f"{inst.name}: too many combined waits on carrier {carrier.name}")
    moved, kept = waits[:-1], waits[-1:]
    if psi is None:
        carrier.sync_info = type(si)(on_wait=moved, on_update=[])
    else:
        psi.on_wait = pwaits + moved
    si.on_wait = kept


def _rebalance_dma_waits(nc):
    """Same single-wait limit applies to HWDGE DMACopy / gpsimd DMA-direct
    instructions. These always read an SBUF tile written by a producer
    (DVE copy / reciprocal) a few instructions earlier; the producer's
    engine tolerates 3 waits, and since the DMA already waits on the
    producer, conditions moved onto the producer still hold when the DMA
    starts."""
    for fn in nc.m.functions:
        for blk in fn.blocks:
            insts = list(blk.instructions)
            sp_seen = {}   # sem name -> max value already awaited on SP queue
            for idx, inst in enumerate(insts):
                if type(inst).__name__ not in (
                        "InstDMACopy", "InstPartitionBroadcast"):
                    continue
                si = inst.sync_info
                waits = list(si.on_wait or []) if si else []
                is_sp = str(inst.engine) == "EngineType.SP"
                if is_sp and waits:
                    # SP executes serially: waits dominated by an earlier SP
                    # instruction's wait on the same sem are redundant
                    live = [w for w in waits
                            if sp_seen.get(w.ant_name, -1) < w.wait_value]
                    if len(live) < len(waits):
                        si.on_wait = live
                        waits = live
                if is_sp:
                    for w in waits:
                        if sp_seen.get(w.ant_name, -1) < w.wait_value:
                            sp_seen[w.ant_name] = w.wait_value
                if len(waits) <= 1:
                    continue
                src = inst.ins[0].memref if inst.ins else None
                prod = None
                for j in range(idx - 1, max(-1, idx - 400), -1):
                    p = insts[j]
                    pouts = getattr(p, "outs", None)
                    if pouts and pouts[0].memref == src and \
                            type(p).__name__ not in ("InstDMACopy",):
                        prod = p
                        break
                if prod is None:
                    # DRAM load: no producer. SP executes serially, so the
                    # nearest preceding wait-free SP DMA can absorb the
                    # engine-WAR wait; the queue wait stays on this DMA.
                    carrier = None
                    for j in range(idx - 1, max(-1, idx - 400), -1):
                        p = insts[j]
                        if type(p).__name__ == "InstDMACopy" and \
                                str(p.engine) == "EngineType.SP":
                            pw = list(p.sync_info.on_wait or []) \
                                if p.sync_info else []
                            if not pw:
                                carrier = p
                                break
                    if carrier is None:
                        # The engine-WAR wait (kept) implies the slot's
                        # previous DMA write completed (its readers waited on
                        # it), so the same-queue WAW wait is redundant.
                        keep = [w for w in waits if "DMAHW" not in w.ant_name]
                        assert len(keep) == 1, (
                            f"{inst.name}: unexpected pair "
                            f"{[(w.ant_name, w.wait_value) for w in waits]}")
                        si.on_wait = keep
                        continue
                    waits.sort(key=lambda w: 1 if "DMAHW" in w.ant_name else 0)
                    _shift_waits(inst, si, waits, carrier)
                    continue
                # keep the producer-engine wait on the DMA, move the rest
                eng = str(prod.engine)
                key = {"EngineType.DVE": "DVE", "EngineType.ACT": "Activation",
                       "EngineType.Pool": "Pool", "EngineType.PE": "PE",
                       "EngineType.SP": "SP"}.get(eng, "zz")
                waits.sort(key=lambda w: 0 if w.ant_name.startswith(key) else 1)
                waits = waits[::-1]  # producer wait last -> kept
                psi = prod.sync_info
                pn = len(list(psi.on_wait or [])) if psi else 0
                if pn + len(waits) - 1 <= 3:
                    _shift_waits(inst, si, waits, prod)
                else:
                    # producer full: queue wait is FIFO-covered (slot reuse
                    # distance is a multiple of the 8 round-robin queues)
                    keep = [w for w in waits if "DMAHW" not in w.ant_name]
                    assert len(keep) == 1, (
                        f"{inst.name}: unexpected {[(w.ant_name, w.wait_value) for w in waits]}")
                    si.on_wait = keep


def _pin_act_tables(nc):
    """The act-table pass picks the first table containing each activation's
    function, so a kernel using Exp and Ln thrashes between table 0
    (exp_and_others) and table 5 (natural_log, which lacks exp) — one
    1.28us ACT table load per job boundary. All functions this kernel uses
    (Exp, Ln, Copy) live together in 'natural_log_exp_and_others', so pin
    the first load to that table and drop the rest (they carry no waits or
    semaphore updates)."""
    from concourse.hw_specs import get_activation_tables

    tables = get_activation_tables(nc.m.arch)
    nl_id = list(tables).index("natural_log_exp_and_others")
    fns = tables["natural_log_exp_and_others"]
    for need in (mybir.ActivationFunctionType.Exp,
                 mybir.ActivationFunctionType.Ln,
                 mybir.ActivationFunctionType.Copy):
        assert need in fns, need
    for fn in nc.m.functions:
        for blk in fn.blocks:
            first = True
            kept = []
            for inst in blk.instructions:
                if isinstance(inst, mybir.InstLoadActFuncSet):
                    si = inst.sync_info
                    assert not (si and (si.on_wait or si.on_update)), inst.name
                    if not first:
                        continue
                    inst.act_func_set_id = nl_id
                    first = False
                kept.append(inst)
            blk.instructions[:] = kept


def build_program(trace_friendly: bool = False):
    nc = bacc.Bacc("TRN2", target_bir_lowering=False)
    ht = nc.dram_tensor("ht", [D, ST], BF16, kind="ExternalInput")
    wq = nc.dram_tensor("wq", [D, QHD], BF16, kind="ExternalInput")
    wkv = nc.dram_tensor("wkv", [D, 2 * HD], BF16, kind="ExternalInput")
    wo = nc.dram_tensor("wo", [QHD, D], BF16, kind="ExternalInput")
    out = nc.dram_tensor("out", [ST, D], BF16, kind="ExternalOutput")

    with tile.TileContext(nc) as tc:
        with (
            tc.tile_pool(name="singles", bufs=1) as singles,
            tc.tile_pool(name="hstream", bufs=3) as hstream,
            tc.tile_pool(name="expp", bufs=4) as expp,
            tc.tile_pool(name="araw", bufs=2) as arawp,
            tc.tile_pool(name="attn", bufs=2) as attnp,
            tc.tile_pool(name="norm", bufs=2) as normp,
            tc.tile_pool(name="ostage", bufs=4) as ostage,
            tc.tile_pool(name="ps_sc", bufs=2, space="PSUM") as ps_sc,
            tc.tile_pool(name="ps_out", bufs=2, space="PSUM") as ps_out,
            tc.tile_pool(name="ps_op", bufs=2, space="PSUM") as ps_op,
        ):
            # ---- resident weights ----
            # Load order matters for the lead-in: wkv (needed by the first
            # matmul) goes first; the first h chunk is DMA'd right after in
            # proj_phase; wq follows; wo is only needed once the first
            # o_proj group runs (one full job later), so it loads last.
            wq_sb = singles.tile([128, DC, QHD], BF16)
            wkv_sb = singles.tile([128, DC, 2 * HD], BF16)
            wo_sb = singles.tile([128, 2, D], BF16)
            nc.sync.dma_start(
                wkv_sb[:, :, :],
                wkv[:, :].rearrange("(a p) j -> p a j", p=128))

            # identity for the PE V-transposes
            id_sb = singles.tile([64, HD], BF16)
            masks.make_identity(nc, id_sb[:, :])

            # ---- resident activations (per batch) ----
            # qtdup[h][b]: [128, S], Q^T duplicated on both partition halves
            qtdup = [[singles.tile([128, S], BF16, tag=f"qt{h}_{b}",
                                   name=f"qt{h}_{b}")
                      for b in range(B)] for h in range(QH)]
            # kt2[b]: [128, KP_N, 128]; rows 0:64 = even kb K^T, 64:128 = odd
            kt2 = [singles.tile([128, KP_N, 128], BF16, tag=f"kt{b}",
                                name=f"kt{b}") for b in range(B)]
            vaug = [singles.tile([128, KB_N, HD + 1], BF16, tag=f"vaug{b}",
                                 name=f"vaug{b}") for b in range(B)]
            # V^T staging: rows 64:128 written by DVE (in-partition from
            # PSUM), rows 0:64 filled by SBUF->SBUF DMA; transposed at the
            # end of the projection phase.
            vt_sb = [singles.tile([128, S], BF16, tag=f"vt{b}", name=f"vt{b}")
                     for b in range(B)]
            for b in range(B):
                nc.vector.memset(vaug[b][:, :, HD:HD + 1], 1.0)

            # pending o_proj work from the previous attention job: a list of
            # closures, each one (2 matmuls + a DVE cast [+ DMA]).
            pending = []

            def run_pending(n):
                for _ in range(min(n, len(pending))):
                    pending.pop(0)()

            def proj_phase(b, pieces=None, split=False):
                """Emit batch-b projection work as ~1.7us closures.

                pieces=None: everything inline. split=True (with pieces):
                KV + Q-heads-0/1 + V-transposes run inline, Q-heads-2/3 are
                appended to pieces (their h chunks are re-loaded, trading
                ~8.4MB of DMA reads to start attention ~20us earlier).
                split=False with pieces: everything is appended."""
                inline = lambda f: f()
                emit = inline if pieces is None else pieces.append
                h_tiles = {}
                pkv_t = {}
                pq_t = {}

                def load(sc, p2=False, b=b):
                    if sc >= SC_N:
                        return
                    h_sb = hstream.tile([128, DC, 512], BF16, tag="h")
                    h_tiles[(p2, sc)] = h_sb
                    scol = b * S + sc * 512
                    # two halves so the first matmuls start at ~1MB loaded
                    for hf in range(2):
                        nc.sync.dma_start(
                            h_sb[:, 8 * hf:8 * hf + 8, :],
                            ht[1024 * hf:1024 * hf + 1024,
                               scol:scol + 512].rearrange(
                                "(a p) j -> p a j", p=128))

                def kv_a(sc, b=b):
                    load(sc + 1)
                    h_sb = h_tiles[(False, sc)]
                    pkv = ps_op.tile([128, 512], F32, tag="po")
                    pkv_t[sc] = pkv
                    for dc in range(8):
                        nc.tensor.matmul(pkv, wkv_sb[:, dc], h_sb[:, dc],
                                         start=(dc == 0), stop=False)

                def kv_b(sc, b=b):
                    h_sb = h_tiles[(False, sc)]
                    pkv = pkv_t.pop(sc)
                    scol = sc * 512
                    for dc in range(8, DC):
                        nc.tensor.matmul(pkv, wkv_sb[:, dc], h_sb[:, dc],
                                         start=False, stop=(dc == DC - 1))
                    # K^T rows 0:64 -> kt2: kb 4sc+j; even j -> low half
                    # (in-partition), odd j -> high half (cross-partition)
                    for j in range(4):
                        kp, half = (4 * sc + j) // 2, (4 * sc + j) % 2
                        nc.vector.tensor_copy(
                            kt2[b][64 * half:64 * half + 64, kp, :],
                            pkv[0:64, j * 128:(j + 1) * 128])
                    # V^T rows 64:128 -> staging high half, then DMA down
                    nc.vector.tensor_copy(
                        vt_sb[b][64:128, scol:scol + 512], pkv[64:128, :])
                    nc.sync.dma_start(
                        vt_sb[b][0:64, scol:scol + 512],
                        vt_sb[b][64:128, scol:scol + 512])

                def q_a(sc, m, p2, b=b):
                    h_sb = h_tiles[(p2, sc)]
                    pq = ps_op.tile([128, 512], F32, tag="po")
                    pq_t[(sc, m)] = pq
                    for dc in range(8):
                        nc.tensor.matmul(
                            pq, wq_sb[:, dc, m * 128:(m + 1) * 128],
                            h_sb[:, dc], start=(dc == 0), stop=False)

                def q_b(sc, m, p2, b=b):
                    scol = sc * 512
                    h_sb = h_tiles[(p2, sc)]
                    if p2:
                        load(sc + 2, p2=True)
                    pq = pq_t.pop((sc, m))
                    for dc in range(8, DC):
                        nc.tensor.matmul(
                            pq, wq_sb[:, dc, m * 128:(m + 1) * 128],
                            h_sb[:, dc], start=False, stop=(dc == DC - 1))
                    h0, h1 = 2 * m, 2 * m + 1
                    nc.vector.tensor_copy(
                        qtdup[h0][b][0:64, scol:scol + 512], pq[0:64, :])
                    nc.vector.tensor_copy(
                        qtdup[h1][b][64:128, scol:scol + 512], pq[64:128, :])
                    # duplicate onto the other partition half (DMA)
                    nc.sync.dma_start(
                        qtdup[h0][b][64:128, scol:scol + 512],
                        qtdup[h0][b][0:64, scol:scol + 512])
                    nc.sync.dma_start(
                        qtdup[h1][b][0:64, scol:scol + 512],
                        qtdup[h1][b][64:128, scol:scol + 512])

                def vtrans(g, b=b):
                    tr = ps_op.tile([128, 4, HD], BF16, tag="po")
                    for j in range(4):
                        kb = 4 * g + j
                        nc.tensor.transpose(
                            tr[:, j, :],
                            vt_sb[b][0:64, kb * 128:(kb + 1) * 128],
                            id_sb[:, :])
                    nc.vector.tensor_copy(
                        vaug[b][:, 4 * g:4 * g + 4, 0:HD], tr[:, :, :])

                load(0)
                if split:
                    # closures returned for event-driven interleave with the
                    # first attention job's heads 0/1
                    assert pieces is not None
                    ev = []
                    for sc in range(SC_N):
                        if sc == SC_N - 1:
                            # fired here (not earlier) so the hstream ring
                            # never makes a pass1 load wait on pass2 readers
                            ev.append(functools.partial(load, 0, True))
                        ev.append(functools.partial(kv_a, sc))
                        ev.append(functools.partial(kv_b, sc))
                        ev.append(functools.partial(q_a, sc, 0, False))
                        ev.append(functools.partial(q_b, sc, 0, False))
                        ev.append(functools.partial(vtrans, sc))
                    # pass2 for sc0 runs inline right after (needed by j0 h2);
                    # the rest are hook pieces
                    ev.append(functools.partial(load, 1, True))
                    ev.append(functools.partial(q_a, 0, 1, True))
                    ev.append(functools.partial(q_b, 0, 1, True))
                    for sc in range(1, SC_N):
                        pieces.append(functools.partial(q_a, sc, 1, True))
                        pieces.append(functools.partial(q_b, sc, 1, True))
                    return ev
                else:
                    for sc in range(SC_N):
                        emit(functools.partial(kv_a, sc))
                        emit(functools.partial(kv_b, sc))
                        for m in range(2):
                            emit(functools.partial(q_a, sc, m, False))
                            emit(functools.partial(q_b, sc, m, False))
                    for g in range(4):
                        emit(functools.partial(vtrans, g))

            # ---------- flat software-pipelined attention ----------
            pieces = []
            cur = {"outp": {}, "araw": None, "attn": None, "expT": {},
                   "hooks": False}

            def emit_scores(s):
                b, qj, h, kp = s
                q0 = qj * 512
                scp = ps_sc.tile([128, 1024], F32)
                nc.tensor.matmul(
                    scp[:, 0:512], kt2[b][0:64, kp, :],
                    qtdup[h][b][0:64, q0:q0 + 512], start=True, stop=True)
                nc.tensor.matmul(
                    scp[:, 512:1024], kt2[b][64:128, kp, :],
                    qtdup[h][b][64:128, q0:q0 + 512], start=True, stop=True)
                expT = expp.tile([128, 1024], BF16)
                nc.scalar.activation(expT[:, :], scp[:, :],
                                     mybir.ActivationFunctionType.Exp,
                                     scale=SCALE)
                cur["expT"][s] = expT

            def finish_head(b, h, araw, attn_sb):
                # per-head normalization, spread through the job so no ACT
                # lump ever stalls the exp stream: 1/den as exp(-ln(den)).
                lnv = normp.tile([1, 512], F32, tag="lnv")
                nc.scalar.activation(lnv, araw[64:65, h, :],
                                     mybir.ActivationFunctionType.Ln)
                recip = normp.tile([1, 512], BF16, tag="recip")
                nc.scalar.activation(recip, lnv,
                                     mybir.ActivationFunctionType.Exp,
                                     scale=-1.0)
                bcast = normp.tile([64, 512], BF16, tag="bcast")
                nc.gpsimd.partition_broadcast(bcast, recip)
                nc.vector.tensor_mul(
                    attn_sb[(h % 2) * 64:(h % 2) * 64 + 64, h // 2, :],
                    araw[0:64, h, :], bcast[:, :])

            def queue_oproj(b, qj, attn_sb, last):
                q0 = qj * 512
                for qc in range(4):
                    ost = ostage.tile([128, 2048], BF16)
                    row = b * S + q0 + qc * 128
                    for nb in range(4):
                        def grp(qc=qc, nb=nb, ost=ost, row=row,
                                attn_sb=attn_sb, act_cast=(last and nb % 2)):
                            po = ps_op.tile([128, 512], F32, tag="po")
                            for hh in range(2):
                                nc.tensor.matmul(
                                    po,
                                    attn_sb[:, hh, qc * 128:(qc + 1) * 128],
                                    wo_sb[:, hh, nb * 512:(nb + 1) * 512],
                                    start=(hh == 0), stop=(hh == 1))
                            if act_cast:
                                nc.scalar.copy(
                                    ost[:, nb * 512:(nb + 1) * 512], po)
                            else:
                                nc.vector.tensor_copy(
                                    ost[:, nb * 512:(nb + 1) * 512], po)
                            if nb == 3:
                                for dd in range(2):
                                    nc.sync.dma_start(
                                        out[row:row + 128,
                                            dd * 1024:(dd + 1) * 1024],
                                        ost[:, dd * 1024:(dd + 1) * 1024])
                        pending.append(grp)

            def emit_pv(s, last):
                b, qj, h, kp = s
                expT = cur["expT"].pop(s)
                if kp == 0:
                    cur["outp"][h] = ps_out.tile([HD + 1, 512], F32,
                                                 name="outp", tag="outp")
                    if h == 0:
                        cur["araw"] = arawp.tile([65, QH, 512], BF16, name="araw", tag="araw")
                outp = cur["outp"][h]
                nc.tensor.matmul(outp, vaug[b][:, 2 * kp, :], expT[:, 0:512],
                                 start=(kp == 0), stop=False)
                nc.tensor.matmul(outp, vaug[b][:, 2 * kp + 1, :],
                                 expT[:, 512:1024],
                                 start=False, stop=(kp == KP_N - 1))
                if kp == KP_N - 1:
                    del cur["outp"][h]
                    # drain this head's accumulator (frees PSUM fast)
                    nc.vector.tensor_copy(cur["araw"][:, h, :], outp)
                    if h == 0:
                        cur["attn"] = attnp.tile([128, 2, 512], BF16,
                                                 name="attn_sb", tag="attn")
                    finish_head(b, h, cur["araw"], cur["attn"])
                    if h == QH - 1:
                        queue_oproj(b, qj, cur["attn"], last)
                        if last:
                            run_pending(len(pending))
                if cur["hooks"]:
                    if kp in (2, 4, 7) and pieces:
                        pieces.pop(0)()
                    if (h == 0 and kp >= 4) or (h > 0 and kp % 2 == 1):
                        run_pending(1)

            # ================= schedule =================
            # wkv + the first h chunk lead; wq follows them; wo (first needed
            # by o_proj of job (0,0), a full job later) loads after proj-b0.
            p1_events = proj_phase(0, pieces=pieces, split=True)
            nc.sync.dma_start(
                wq_sb[:, :, :], wq[:, :].rearrange("(a p) j -> p a j", p=128))

            prev = [None]

            def do_slot(s):
                emit_scores(s)
                if prev[0] is not None:
                    emit_pv(prev[0], last=False)
                prev[0] = s

            # pass1 interleave: per s-chunk, project then run the newly
            # enabled key-block pairs for heads 0 and 1 of job (0,0)
            ei = 0
            for sc in range(SC_N):
                n_ev = 6 if sc == SC_N - 1 else 5
                for _ in range(n_ev):
                    p1_events[ei](); ei += 1
                for kp in (2 * sc, 2 * sc + 1):
                    do_slot((0, 0, 0, kp))
                    do_slot((0, 0, 1, kp))
            for e in p1_events[ei:]:
                e()
            nc.sync.dma_start(
                wo_sb[:, :, :], wo[:, :].rearrange("(a p) j -> p a j", p=128))
            proj_phase(1, pieces=pieces)
            cur["hooks"] = True

            rest = [(0, 0, h, kp) for h in (2, 3) for kp in range(KP_N)]
            rest += [(0, qj, h, kp) for qj in range(1, QJ_N)
                     for h in range(QH) for kp in range(KP_N)]
            rest += [(1, qj, h, kp) for qj in range(QJ_N)
                     for h in range(QH) for kp in range(KP_N)]
            for i, s in enumerate(rest):
                if s[:2] == (1, 0) and s[2:] == (0, 0):
                    assert not pieces, len(pieces)
                do_slot(s)
            emit_pv(prev[0], last=True)
            run_pending(len(pending))
    nc.compile()
    _pin_act_tables(nc)
    _rebalance_matmul_waits(nc)
    _rebalance_dma_waits(nc)
    return nc


@functools.lru_cache(maxsize=1)
def _get_program():
    return build_program()


def _in_maps(hidden_states, Wq, Wk, Wv, Wo):
    bf = ml_dtypes.bfloat16
    htT = np.ascontiguousarray(
        hidden_states.reshape(ST, D).T.astype(bf))          # [D, B*S]
    in_maps = []
    for c in range(NCORES):
        wkv = np.concatenate(
            [Wk[:, c * HD:(c + 1) * HD], Wv[:, c * HD:(c + 1) * HD]], axis=1)
        in_maps.append({
            "ht": htT,
            "wq": np.ascontiguousarray(Wq[:, c * QHD:(c + 1) * QHD].astype(bf)),
            "wkv": np.ascontiguousarray(wkv.astype(bf)),
            "wo": np.ascontiguousarray(Wo[c * QHD:(c + 1) * QHD, :].astype(bf)),
        })
    return in_maps


def kernel(hidden_states, Wq, Wk, Wv, Wo):
    hidden_states = np.asarray(hidden_states)
    Wq, Wk, Wv, Wo = (np.asarray(x) for x in (Wq, Wk, Wv, Wo))
    in_maps = _in_maps(hidden_states, Wq, Wk, Wv, Wo)
    nc = _get_program()
    res = run_bass_kernel_spmd(nc, in_maps, core_ids=list(range(NCORES)))
    total = res.results[0]["out"].astype(np.float64)
    for c in range(1, NCORES):
        total += res.results[c]["out"].astype(np.float64)
    return total.reshape(B, S, D).astype(np.float32)
